# revision 24
# baseline (speedup 1.0000x reference)
"""CloudResourceGNN (2-layer GAT + resource embedding) on 8 Trainium2 NeuronCores.

Layout: nodes sorted by in-degree (desc) and dealt into 128-node blocks;
global block g -> core g%8, slot g//8, partition = n%128. Edges grouped by
dst block; slot (tile i, partition p) holds the i-th edge (src-sorted) of
dst node p in the block, padded to the block's max degree. With partition ==
dst: a_dst is a per-partition Activation-engine bias, the softmax scatter is
an identity-lhsT PSUM accumulate, and no per-edge dst-side gathers exist.
Src rows are fetched with one SWDGE dma_gather per (layer, block) from
pair-packed tables (1024B rows L1 / 512B L2, idx = src>>1, int16-safe);
parity is resolved by dual masked matmuls. Softmax runs without
max-subtraction: w = exp(leaky_relu(asrc+adst)) with denominators from ones
columns. LayerNorm runs on the Activation engine via ln/exp (one act table).
The graph-independent resource-embedding half of the output is computed and
written during the layer-1 edge phase.
"""

import numpy as np
import ml_dtypes

import concourse.bass as bass
import concourse.bacc as bacc
import concourse.mybir as mybir
import concourse.tile as tile

BF16 = mybir.dt.bfloat16
F32 = mybir.dt.float32
I16 = mybir.dt.int16
OPc = mybir.AluOpType
AF = mybir.ActivationFunctionType
nbf = ml_dtypes.bfloat16

NEG_SLOPE = 0.2
LN_EPS = 1e-5
P = 128


class Geo:
    pass


def _wrap16(vals):
    """idx list (len % 128 == 0) -> [128, n/16] wrapped-16, replicated x8."""
    v = np.asarray(vals, np.int64)
    assert len(v) % 128 == 0
    w = v.reshape(-1, 16).T                    # [16, n/16]
    return np.tile(w, (8, 1)).astype(np.int16)  # [128, n/16]


def build_geometry(N, n_cores, src, dst, cap=20):
    g = Geo()
    g.N = N
    g.n_cores = n_cores
    per_core_nodes = -(-N // n_cores)
    g.nblk = -(-per_core_nodes // P)
    g.npc = g.nblk * P
    g.node_pad = g.npc * n_cores
    g.ntile = g.node_pad // P          # global blocks

    loop = np.arange(N, dtype=np.int64)
    s_all = np.concatenate([np.asarray(src, np.int64), loop])
    d_all = np.concatenate([np.asarray(dst, np.int64), loop])
    deg = np.bincount(d_all, minlength=N)

    # degree-sorted placement: new id n = rank in descending-degree order
    g.order = np.argsort(-deg, kind="stable")          # new -> old
    pos = np.empty(N, np.int64)
    pos[g.order] = np.arange(N)                        # old -> new
    g.pos = pos

    sn = pos[s_all]
    dn = pos[d_all]
    gblk = dn // P                                     # global dst block
    core = gblk % n_cores
    lblk = gblk // n_cores                             # local block slot
    part = dn % P

    # tiles per local-block slot j: max over cores/partitions of per-node deg
    degn = np.zeros(g.node_pad, np.int64)
    degn[:N] = deg[g.order]
    dmax = degn.reshape(g.ntile, P).max(axis=1)        # per global block
    g.Tb = np.zeros(g.nblk, np.int64)
    for j in range(g.nblk):
        g.Tb[j] = max(1, dmax[j * n_cores:(j + 1) * n_cores].max())
    g.T = int(g.Tb.sum())
    g.S = g.T * P
    g.t0 = np.zeros(g.nblk, np.int64)
    g.t0[1:] = np.cumsum(g.Tb)[:-1]
    g.gmax = min(cap, int(g.Tb.max()))
    # chunks: (block, tile_lo, tile_hi, first, last) with tile_hi-tile_lo <= cap
    g.chunks = []
    for j in range(g.nblk):
        lo = 0
        while lo < g.Tb[j]:
            hi = min(lo + cap, int(g.Tb[j]))
            g.chunks.append((j, lo, hi, lo == 0, hi == g.Tb[j]))
            lo = hi

    # layer-2 global row of node n (AllGather order: core-major)
    def t2row(n):
        gb = n // P
        return (gb % n_cores) * g.npc + (gb // n_cores) * P + n % P

    g.ix1 = np.zeros((n_cores, P, g.S // 16), np.int16)
    g.ix2 = np.zeros((n_cores, P, g.S // 16), np.int16)
    g.jm = np.zeros((n_cores, P, 2 * g.T), np.float32)
    g.ownmask = np.zeros((n_cores, P, g.ntile), np.float32)

    for k in range(n_cores):
        m = core == k
        s, j, p = sn[m], lblk[m], part[m]
        # sort edges by (block, partition, src) so slot i of node p is its
        # i-th smallest src (quantile locality across partitions)
        o = np.lexsort((s, p, j))
        s, j, p = s[o], j[o], p[o]
        v1 = np.zeros(g.S, np.int64)
        v2 = np.zeros(g.S, np.int64)
        pme = np.zeros(g.S, np.float32)
        pmo = np.zeros(g.S, np.float32)
        vld = np.zeros(g.S, bool)
        # slot index: i-th edge of (j, p) -> (g.t0[j] + i) * P + p
        # compute i via cumcount within (j, p) runs
        if len(s):
            key = j * P + p
            start = np.r_[0, np.nonzero(np.diff(key))[0] + 1]
            runlen = np.diff(np.r_[start, len(key)])
            within = np.arange(len(key)) - np.repeat(start, runlen)
            slot = (g.t0[j] + within) * P + p
            v1[slot] = s >> 1
            v2[slot] = t2row(s) >> 1
            vld[slot] = True
            even = (s % 2 == 0).astype(np.float32)
            pme[slot] = even
            pmo[slot] = 1.0 - even
        # trailing-pad slots per chunk -> idx -1 (Q7 trims them)
        for (j2, tl, th, _, _) in g.chunks:
            a, b = (g.t0[j2] + tl) * P, (g.t0[j2] + th) * P
            e = b
            while e > a and not vld[e - 1]:
                e -= 1
            g.ix1[k, :, a // 16:b // 16] = _wrap16(v1[a:b])
            g.ix2[k, :, a // 16:b // 16] = _wrap16(v2[a:b])
        jm = np.stack([pme, pmo], axis=-1).reshape(g.T, P, 2)
        g.jm[k] = jm.transpose(1, 0, 2).reshape(P, 2 * g.T)
        own = np.zeros(g.ntile, np.float32)
        own[np.arange(g.ntile) % n_cores == k] = 1.0
        g.ownmask[k] = np.tile(own, (P, 1))
    return g


def pack_weights(W1, att_src1, att_dst1, W2, att_src2, att_dst2, hid, heads):
    C1 = W1.shape[0]
    n1 = 2 * (hid + 1) + 2 * heads      # 134: [h0|1|h1|1|as0,as1|ad0,ad1]
    rhs1 = np.zeros((C1, n1), dtype=np.float32)
    rhs1[:, 0:hid] = W1[:, 0:hid]
    rhs1[:, hid + 1:2 * hid + 1] = W1[:, hid:2 * hid]
    Wh = W1.reshape(C1, heads, hid)
    rhs1[:, 2 * hid + 2:2 * hid + 2 + heads] = np.einsum("ihc,hc->ih", Wh, att_src1)
    rhs1[:, 2 * hid + 2 + heads:] = np.einsum("ihc,hc->ih", Wh, att_dst1)
    ones1 = np.zeros((1, n1), dtype=np.float32)
    ones1[0, hid] = 1.0
    ones1[0, 2 * hid + 1] = 1.0
    C2 = W2.shape[0]
    n2 = hid + 3                        # 67: [h|1|asrc2|adst2]
    rhs2 = np.zeros((C2, n2), dtype=np.float32)
    rhs2[:, 0:hid] = W2
    rhs2[:, hid + 1] = W2 @ att_src2[0]
    rhs2[:, hid + 2] = W2 @ att_dst2[0]
    ones2 = np.zeros((1, n2), dtype=np.float32)
    ones2[0, hid] = 1.0
    return rhs1, ones1, rhs2, ones2


def build_program(g, hid=64, heads=2, C1=128, R=16, res_dim=64):
    NT = g.ntile
    NB = g.nblk
    n1 = 2 * (hid + 1) + 2 * heads      # 134
    n2 = hid + 3                        # 67
    w1c = hid + 1                       # 65
    T1C = 256                           # table1 cols per node (bf16, 512B)
    T2C = 128                           # table2 cols per node (bf16, 256B)
    RROW = NB * P * R

    nc = bacc.Bacc("TRN2", target_bir_lowering=False, debug=False,
                   num_devices=g.n_cores)

    xT_bf = nc.dram_tensor("xT_bf", [C1, g.node_pad], BF16, kind="ExternalInput")
    rhs1_d = nc.dram_tensor("rhs1", [C1, n1], BF16, kind="ExternalInput")
    ones1_d = nc.dram_tensor("ones1row", [1, n1], BF16, kind="ExternalInput")
    rhs2_d = nc.dram_tensor("rhs2", [C1, n2], BF16, kind="ExternalInput")
    ones2_d = nc.dram_tensor("ones2row", [1, n2], BF16, kind="ExternalInput")
    resw_d = nc.dram_tensor("resw", [res_dim + 1, hid], BF16, kind="ExternalInput")
    consts_d = nc.dram_tensor("consts", [8, 512], F32, kind="ExternalInput")
    ident_d = nc.dram_tensor("ident", [P, P], BF16, kind="ExternalInput")
    resT_d = nc.dram_tensor("resT_bf", [res_dim, RROW], BF16, kind="ExternalInput")
    ix1_d = nc.dram_tensor("ix1", [P, g.S // 16], I16, kind="ExternalInput")
    ix2_d = nc.dram_tensor("ix2", [P, g.S // 16], I16, kind="ExternalInput")
    jm_d = nc.dram_tensor("jm", [P, 2 * g.T], F32, kind="ExternalInput")
    own_d = nc.dram_tensor("ownmask", [P, NT], F32, kind="ExternalInput")
    out_d = nc.dram_tensor("out", [g.npc, R, 2 * hid], F32, kind="ExternalOutput")

    table1 = nc.dram_tensor("table1", [g.node_pad, T1C], BF16)
    myblk2 = nc.dram_tensor("myblk2", [g.npc, T2C], BF16)
    table2 = nc.dram_tensor("table2", [g.n_cores * g.npc, T2C], BF16,
                            addr_space="Shared")
    t1pair = table1.ap().rearrange("(r x) c -> r (x c)", x=2)   # [np/2, 512]
    t2pair = table2.ap().rearrange("(r x) c -> r (x c)", x=2)   # [np*8/2? -> 512]

    with tile.TileContext(nc) as tc:
        with tc.tile_pool(name="consts", bufs=1) as cpool, \
             tc.tile_pool(name="persist", bufs=1) as pp:
            crow = []
            for r in range(8):
                t_ = cpool.tile([1, 512], F32, tag=f"crow{r}", name=f"crow{r}")
                nc.sync.dma_start(out=t_[:, :], in_=consts_d[r:r + 1, :])
                crow.append(t_)
            onesbf = cpool.tile([1, P], BF16)
            nc.vector.tensor_copy(out=onesbf[:, :], in_=crow[7][:, 0:P])
            eps_t = cpool.tile([P, 1], F32)
            nc.vector.memset(eps_t[:, :], LN_EPS)
            grd_t = cpool.tile([P, 1], F32)
            nc.vector.memset(grd_t[:, :], 1e-20)
            ident_s = cpool.tile([P, P], BF16)
            nc.sync.dma_start(out=ident_s[:, :], in_=ident_d[:, :])
            rhs1_s = cpool.tile([C1, n1], BF16)
            nc.sync.dma_start(out=rhs1_s[:, :], in_=rhs1_d[:, :])
            ones1_s = cpool.tile([1, n1], BF16)
            nc.sync.dma_start(out=ones1_s[:, :], in_=ones1_d[:, :])
            rhs2_s = cpool.tile([C1, n2], BF16)
            nc.sync.dma_start(out=rhs2_s[:, :], in_=rhs2_d[:, :])
            ones2_s = cpool.tile([1, n2], BF16)
            nc.sync.dma_start(out=ones2_s[:, :], in_=ones2_d[:, :])
            resw_s = cpool.tile([res_dim + 1, hid], BF16)
            nc.sync.dma_start(out=resw_s[:, :], in_=resw_d[:, :])

            ones_f = cpool.tile([1, P], F32)
            nc.vector.tensor_copy(out=ones_f[:, :], in_=crow[7][:, 0:P])
            b1_rep = cpool.tile([P, 2 * hid], F32)
            b2_rep = cpool.tile([P, hid], F32)
            lnw_rep = cpool.tile([P, hid], F32)
            lnb_rep = cpool.tile([P, hid], F32)
            with tc.tile_pool(name="repl_ps", bufs=2, space="PSUM") as rps:
                for dst_t, row, ncol in (
                    (b1_rep, 0, 2 * hid), (b2_rep, 1, hid),
                    (lnw_rep, 2, hid), (lnb_rep, 3, hid),
                ):
                    pst = rps.tile([P, 512], F32, tag="repl", name=f"repl{row}")
                    nc.tensor.matmul(out=pst[:, 0:ncol], lhsT=ones_f[:, :],
                                     rhs=crow[row][:, 0:ncol],
                                     start=True, stop=True)
                    nc.vector.tensor_copy(out=dst_t[:, 0:ncol],
                                          in_=pst[:, 0:ncol])

            jm_sb = pp.tile([P, g.T, 2], F32)
            nc.sync.dma_start(out=jm_sb[:, :, :], in_=jm_d[:, :])
            ix1_sb = pp.tile([P, g.S // 16], I16)
            nc.sync.dma_start(out=ix1_sb[:, :], in_=ix1_d[:, :])
            ix2_sb = pp.tile([P, g.S // 16], I16)
            nc.sync.dma_start(out=ix2_sb[:, :], in_=ix2_d[:, :])
            ownm = pp.tile([P, NT], F32)
            nc.sync.dma_start(out=ownm[:, :], in_=own_d[:, :])
            blk2_sb = pp.tile([P, NB, T2C], BF16)
            adst1_sb = pp.tile([P, NB, heads], F32)
            adst2_sb = pp.tile([P, NB, 1], F32)
            nc.vector.memset(adst1_sb[:, :, :], 0.0)
            nc.vector.memset(adst2_sb[:, :, :], 0.0)
            nc.vector.memset(blk2_sb[:, :, :], 0.0)

            # ---------------- phase 1: node phase (replicated) -------------
            XCH = 32
            with tc.tile_pool(name="n1_xt", bufs=2) as xtp, \
                 tc.tile_pool(name="n1_ps", bufs=4, space="PSUM") as n1ps, \
                 tc.tile_pool(name="n1_st", bufs=3) as n1st:
                nch = -(-NT // XCH)
                sb_iter = 0
                for c in range(nch):
                    tn0 = c * XCH
                    ntl = min(XCH, NT - tn0)
                    xt = xtp.tile([P, XCH * P], BF16, tag="xt")
                    nc.sync.dma_start(out=xt[:, 0:ntl * P],
                                      in_=xT_bf[:, tn0 * P:(tn0 + ntl) * P])
                    nst = -(-ntl // 4)
                    for sb in range(nst):
                        st = n1st.tile([P, 4, T1C], BF16, tag="n1st")
                        nn = min(4, ntl - sb * 4)
                        if sb_iter < 3:
                            nc.vector.memset(st[:, :, n1 - 2:T1C], 0.0)
                        sb_iter += 1
                        for i in range(nn):
                            t = sb * 4 + i
                            gt_ = tn0 + t
                            b = gt_ // g.n_cores
                            ps = n1ps.tile([P, n1], F32, tag="n1ps")
                            nc.tensor.matmul(out=ps[:, :],
                                             lhsT=xt[:, t * P:(t + 1) * P],
                                             rhs=rhs1_s[:, :],
                                             start=True, stop=False)
                            nc.tensor.matmul(out=ps[:, :], lhsT=onesbf[:, :],
                                             rhs=ones1_s[:, :],
                                             start=False, stop=True)
                            nc.scalar.copy(out=st[:, i:i + 1, 0:n1 - 2],
                                           in_=ps[:, 0:n1 - 2])
                            nc.vector.scalar_tensor_tensor(
                                out=adst1_sb[:, b:b + 1, 0:heads],
                                in0=ps[:, n1 - 2:n1],
                                scalar=ownm[:, gt_:gt_ + 1],
                                in1=adst1_sb[:, b:b + 1, 0:heads],
                                op0=OPc.mult, op1=OPc.add)
                        nc.sync.dma_start(
                            out=table1.ap().rearrange(
                                "(t p) c -> p t c",
                                p=P)[:, tn0 + sb * 4:tn0 + sb * 4 + nn, :],
                            in_=st[:, 0:nn, :])

            # ------- phase 2: layer-1 edge phase + res embedding -----------
            with tc.tile_pool(name="e1_g", bufs=3) as gp, \
                 tc.tile_pool(name="e1_w", bufs=2) as wp, \
                 tc.tile_pool(name="e1_ps", bufs=2, space="PSUM") as eps, \
                 tc.tile_pool(name="e1_tp", bufs=1, space="PSUM") as tps, \
                 tc.tile_pool(name="e1_h2", bufs=1, space="PSUM") as h2ps, \
                 tc.tile_pool(name="e1_x2", bufs=2) as x2p, \
                 tc.tile_pool(name="res_t", bufs=2) as resp, \
                 tc.tile_pool(name="res_ps", bufs=2, space="PSUM") as rps2:
                res_iter = 0
                g_iter = 0
                psum_cur = {}
                for ci, (b, tl, th, first, last) in enumerate(g.chunks):
                    t0 = int(g.t0[b]) + tl
                    ntl = th - tl
                    gt = gp.tile([P, g.gmax, 2 * T1C], BF16, tag="g1",
                                 name=f"g1_{ci}")
                    if g_iter < 3:
                        nc.vector.memset(gt[:, :, :], 0.0)
                    g_iter += 1
                    nc.gpsimd.dma_gather(
                        gt[:, 0:ntl, :], t1pair,
                        ix1_sb[:, t0 * 8:(t0 + ntl) * 8], ntl * P, ntl * P,
                        2 * T1C, single_packet=False)
                    # w = exp(leaky(asrc + adst)) per (parity, head)
                    wt = wp.tile([P, g.gmax, 4], F32, tag="w1", name=f"w1_{ci}")
                    for par in range(2):
                        for h in range(heads):
                            c = par * heads + h
                            ac = par * T1C + n1 - 4 + h
                            nc.scalar.activation(
                                out=wt[:, 0:ntl, c:c + 1],
                                in_=gt[:, 0:ntl, ac:ac + 1],
                                func=AF.Identity,
                                bias=adst1_sb[:, b, h:h + 1])
                    nc.vector.scalar_tensor_tensor(
                        out=wt[:, 0:ntl, :], in0=wt[:, 0:ntl, :],
                        scalar=NEG_SLOPE, in1=wt[:, 0:ntl, :],
                        op0=OPc.mult, op1=OPc.max)
                    nc.scalar.activation(out=wt[:, 0:ntl, :],
                                         in_=wt[:, 0:ntl, :], func=AF.Exp)
                    nc.vector.tensor_tensor(
                        out=wt[:, 0:ntl, 0:2], in0=wt[:, 0:ntl, 0:2],
                        in1=jm_sb[:, t0:t0 + ntl, 0:1].to_broadcast(
                            [P, ntl, 2]), op=OPc.mult)
                    nc.vector.tensor_tensor(
                        out=wt[:, 0:ntl, 2:4], in0=wt[:, 0:ntl, 2:4],
                        in1=jm_sb[:, t0:t0 + ntl, 1:2].to_broadcast(
                            [P, ntl, 2]), op=OPc.mult)
                    gs = gp.tile([P, g.gmax, 4 * w1c], BF16, tag="gs",
                                 name=f"gs_{ci}")
                    for par in range(2):
                        for h in range(heads):
                            nc.vector.tensor_tensor(
                                out=gs[:, 0:ntl,
                                       (par * heads + h) * w1c:
                                       (par * heads + h + 1) * w1c],
                                in0=gt[:, 0:ntl,
                                       par * T1C + h * w1c:
                                       par * T1C + (h + 1) * w1c],
                                in1=wt[:, 0:ntl,
                                       par * heads + h:par * heads + h + 1
                                       ].to_broadcast([P, ntl, w1c]),
                                op=OPc.mult)
                    if first:
                        psum_cur[b] = eps.tile([P, heads * w1c], F32,
                                               tag="e1ps", name=f"e1ps_{b}")
                    pc = psum_cur[b]
                    for i in range(ntl):
                        for par in range(2):
                            nc.tensor.matmul(
                                out=pc[:, :], lhsT=ident_s[:, :],
                                rhs=gs[:, i:i + 1,
                                       par * heads * w1c:
                                       (par + 1) * heads * w1c],
                                start=(first and i == 0 and par == 0),
                                stop=(last and i == ntl - 1 and par == 1))
                    if not last:
                        continue
                    pc = psum_cur.pop(b)
                    # epilogue: softmax div + bias + ELU -> x2t
                    x2pre = x2p.tile([P, 2 * hid], F32, tag="x2pre",
                                     name=f"x2pre_{b}")
                    esc = x2p.tile([P, 2 * hid], F32, tag="esc",
                                   name=f"esc_{b}")
                    x2t = x2p.tile([P, 2 * hid], BF16, tag="x2",
                                   name=f"x2_{b}")
                    for h in range(heads):
                        rec = x2p.tile([P, 1], F32, tag=f"rec{h}",
                                       name=f"rec{h}_{b}")
                        dn = x2p.tile([P, 1], F32, tag=f"dn{h}",
                                      name=f"dn{h}_{b}")
                        nc.scalar.activation(
                            out=dn[:, :],
                            in_=pc[:, (h + 1) * w1c - 1:(h + 1) * w1c],
                            func=AF.Identity, bias=grd_t[:, 0:1])
                        nc.vector.reciprocal(out=rec[:, :], in_=dn[:, :])
                        nc.vector.scalar_tensor_tensor(
                            out=x2pre[:, h * hid:(h + 1) * hid],
                            in0=pc[:, h * w1c:h * w1c + hid],
                            scalar=rec[:, 0:1],
                            in1=b1_rep[:, h * hid:(h + 1) * hid],
                            op0=OPc.mult, op1=OPc.add)
                    # ELU: x2t = max(exp(min(x,0)) - 1, x)
                    nc.scalar.activation(out=esc[:, :], in_=x2pre[:, :],
                                         func=AF.Relu, scale=-1.0)
                    nc.scalar.activation(out=esc[:, :], in_=esc[:, :],
                                         func=AF.Exp, scale=-1.0)
                    nc.vector.scalar_tensor_tensor(
                        out=x2t[:, :], in0=esc[:, :], scalar=-1.0,
                        in1=x2pre[:, :], op0=OPc.add, op1=OPc.max)
                    tp = tps.tile([P, P], BF16, tag="x2tp", name=f"tp_{b}")
                    nc.tensor.transpose(out=tp[:, :], in_=x2t[:, :],
                                        identity=ident_s[:, :])
                    x2tt = x2p.tile([P, P], BF16, tag="x2tt", name=f"x2tt_{b}")
                    nc.scalar.copy(out=x2tt[:, :], in_=tp[:, :])
                    h2 = h2ps.tile([P, n2], F32, tag="h2ps", name=f"h2_{b}")
                    nc.tensor.matmul(out=h2[:, :], lhsT=x2tt[:, :],
                                     rhs=rhs2_s[:, :], start=True, stop=False)
                    nc.tensor.matmul(out=h2[:, :], lhsT=onesbf[:, :],
                                     rhs=ones2_s[:, :], start=False, stop=True)
                    nc.scalar.copy(out=blk2_sb[:, b:b + 1, 0:n2 - 1],
                                   in_=h2[:, 0:n2 - 1])
                    nc.vector.tensor_copy(out=adst2_sb[:, b:b + 1, 0:1],
                                          in_=h2[:, n2 - 1:n2])

                    # res embedding for this block (graph-independent)
                    rt = resp.tile([res_dim + 1, P, R], BF16, tag="rest",
                                   name=f"rt_{b}")
                    if res_iter < 2:
                        nc.vector.memset(rt[res_dim:res_dim + 1, :, :], 1.0)
                    res_iter += 1
                    nc.sync.dma_start(
                        out=rt[0:res_dim, :, :],
                        in_=resT_d[:, b * P * R:(b + 1) * P * R])
                    for half in range(2):
                        rp = rps2.tile([P, 8 * hid], F32, tag="resps",
                                       name=f"rp_{b}_{half}")
                        for r8 in range(8):
                            r = half * 8 + r8
                            nc.tensor.matmul(
                                out=rp[:, r8 * hid:(r8 + 1) * hid],
                                lhsT=rt[:, :, r:r + 1],
                                rhs=resw_s[:, :], start=True, stop=True)
                        em = resp.tile([P, 8, hid], F32, tag="em",
                                       name=f"em_{b}_{half}")
                        ro = resp.tile([P, 8, hid], F32, tag="ro",
                                       name=f"ro_{b}_{half}")
                        nc.scalar.activation(out=em[:, :, :], in_=rp[:, :],
                                             func=AF.Relu, scale=-1.0)
                        nc.scalar.activation(out=em[:, :, :], in_=em[:, :, :],
                                             func=AF.Exp, scale=-1.0)
                        nc.vector.scalar_tensor_tensor(
                            out=ro[:, :, :], in0=em[:, :, :], scalar=-1.0,
                            in1=rp[:, :], op0=OPc.add, op1=OPc.max)
                        nc.sync.dma_start(
                            out=out_d[b * P:(b + 1) * P,
                                      half * 8:(half + 1) * 8, hid:2 * hid],
                            in_=ro[:, :, :])

            nc.sync.dma_start(
                out=myblk2.ap().rearrange("(j p) c -> p j c", p=P)[:, :, :],
                in_=blk2_sb[:, :, :])
            nc.gpsimd.collective_compute(
                "AllGather", OPc.bypass,
                replica_groups=[list(range(g.n_cores))],
                ins=[myblk2.ap().opt()],
                outs=[table2.ap().opt()],
            )

            # -------- phase 3: layer-2 edge phase + LN + output ------------
            GRP = 8
            with tc.tile_pool(name="e2_g", bufs=3) as gp2, \
                 tc.tile_pool(name="e2_w", bufs=2) as wp2, \
                 tc.tile_pool(name="e2_ps", bufs=2, space="PSUM") as eps2, \
                 tc.tile_pool(name="ln", bufs=2) as lnp, \
                 tc.tile_pool(name="lng", bufs=2) as lgp:
                g2_iter = 0
                psum2 = {}
                xcg = None
                for ci, (b, tl, th, first, last) in enumerate(g.chunks):
                    t0 = int(g.t0[b]) + tl
                    ntl = th - tl
                    gt2 = gp2.tile([P, g.gmax, 2 * T2C], BF16, tag="g2",
                                   name=f"g2_{ci}")
                    if g2_iter < 3:
                        nc.vector.memset(gt2[:, :, :], 0.0)
                    g2_iter += 1
                    nc.gpsimd.dma_gather(
                        gt2[:, 0:ntl, :], t2pair,
                        ix2_sb[:, t0 * 8:(t0 + ntl) * 8], ntl * P, ntl * P,
                        2 * T2C, single_packet=False)
                    wt2 = wp2.tile([P, g.gmax, 2], F32, tag="w2",
                                   name=f"w2_{ci}")
                    for par in range(2):
                        ac = par * T2C + hid + 1
                        nc.scalar.activation(
                            out=wt2[:, 0:ntl, par:par + 1],
                            in_=gt2[:, 0:ntl, ac:ac + 1],
                            func=AF.Identity,
                            bias=adst2_sb[:, b, 0:1])
                    nc.vector.scalar_tensor_tensor(
                        out=wt2[:, 0:ntl, :], in0=wt2[:, 0:ntl, :],
                        scalar=NEG_SLOPE, in1=wt2[:, 0:ntl, :],
                        op0=OPc.mult, op1=OPc.max)
                    nc.scalar.activation(out=wt2[:, 0:ntl, :],
                                         in_=wt2[:, 0:ntl, :], func=AF.Exp)
                    nc.vector.tensor_tensor(
                        out=wt2[:, 0:ntl, :], in0=wt2[:, 0:ntl, :],
                        in1=jm_sb[:, t0:t0 + ntl, :], op=OPc.mult)
                    gs2 = gp2.tile([P, g.gmax, 2 * w1c], BF16, tag="gs2",
                                   name=f"gs2_{ci}")
                    for par in range(2):
                        nc.vector.tensor_tensor(
                            out=gs2[:, 0:ntl, par * w1c:(par + 1) * w1c],
                            in0=gt2[:, 0:ntl, par * T2C:par * T2C + w1c],
                            in1=wt2[:, 0:ntl, par:par + 1].to_broadcast(
                                [P, ntl, w1c]),
                            op=OPc.mult)
                    if first:
                        psum2[b] = eps2.tile([P, w1c], F32, tag="e2ps",
                                             name=f"e2ps_{b}")
                    ps2 = psum2[b]
                    for i in range(ntl):
                        for par in range(2):
                            nc.tensor.matmul(
                                out=ps2[:, :], lhsT=ident_s[:, :],
                                rhs=gs2[:, i:i + 1,
                                        par * w1c:(par + 1) * w1c],
                                start=(first and i == 0 and par == 0),
                                stop=(last and i == ntl - 1 and par == 1))
                    if not last:
                        continue
                    ps2 = psum2.pop(b)
                    jg = b % GRP
                    if jg == 0:
                        xcg = lgp.tile([P, GRP, hid], F32, tag="xcg",
                                       name=f"xcg_{b}")
                        mvg = lgp.tile([P, GRP, 2], F32, tag="mvg",
                                       name=f"mvg_{b}")
                        sdg = lgp.tile([P, GRP], F32, tag="sdg",
                                       name=f"sdg_{b}")
                        rsg = lgp.tile([P, GRP], F32, tag="rsg",
                                       name=f"rsg_{b}")
                    # softmax div + bias -> y; mean/var on DVE (bn_stats)
                    y = lnp.tile([P, hid], F32, tag="y", name=f"y_{b}")
                    rec = lnp.tile([P, 1], F32, tag="rec2", name=f"r2_{b}")
                    nmu = lnp.tile([P, 1], F32, tag="nmu", name=f"n2_{b}")
                    st6 = lnp.tile([P, 6], F32, tag="st6", name=f"s6_{b}")
                    dn2 = lnp.tile([P, 1], F32, tag="dn2", name=f"d2_{b}")
                    nc.scalar.activation(out=dn2[:, :],
                                         in_=ps2[:, hid:hid + 1],
                                         func=AF.Identity,
                                         bias=grd_t[:, 0:1])
                    nc.vector.reciprocal(out=rec[:, :], in_=dn2[:, :])
                    nc.vector.scalar_tensor_tensor(
                        out=y[:, :], in0=ps2[:, 0:hid], scalar=rec[:, 0:1],
                        in1=b2_rep[:, :], op0=OPc.mult, op1=OPc.add)
                    nc.vector.bn_stats(out=st6[:, :], in_=y[:, :])
                    nc.vector.bn_aggr(out=mvg[:, jg, :], in_=st6[:, :])
                    nc.scalar.activation(out=nmu[:, :],
                                         in_=mvg[:, jg, 0:1],
                                         func=AF.Identity, scale=-1.0)
                    nc.scalar.activation(out=xcg[:, jg, :], in_=y[:, :],
                                         func=AF.Identity,
                                         bias=nmu[:, 0:1])
                    if jg == GRP - 1 or b == NB - 1:
                        gn = jg + 1
                        b0 = b - jg
                        nc.scalar.activation(out=sdg[:, 0:gn],
                                             in_=mvg[:, 0:gn, 1],
                                             func=AF.Sqrt,
                                             bias=eps_t[:, 0:1])
                        nc.vector.reciprocal(out=rsg[:, 0:gn],
                                             in_=sdg[:, 0:gn])
                        for j2 in range(gn):
                            bb = b0 + j2
                            lnh = lnp.tile([P, 1, hid], F32, tag="lnh",
                                           name=f"lnh_{bb}")
                            nc.vector.scalar_tensor_tensor(
                                out=lnh[:, 0, :], in0=xcg[:, j2, :],
                                scalar=rsg[:, j2:j2 + 1],
                                in1=lnw_rep[:, :],
                                op0=OPc.mult, op1=OPc.mult)
                            nc.vector.tensor_tensor(out=lnh[:, 0, :],
                                                    in0=lnh[:, 0, :],
                                                    in1=lnb_rep[:, :],
                                                    op=OPc.add)
                            nc.sync.dma_start(
                                out=out_d[bb * P:(bb + 1) * P, :, 0:hid],
                                in_=lnh[:, 0:1, :].to_broadcast(
                                    [P, R, hid]))
    nc.compile()
    return nc


# ----------------------------------------------------------------------------
# host wrapper
# ----------------------------------------------------------------------------

def make_inputs(g, x, resource_features, W1, att_src1, att_dst1, b1,
                W2, att_src2, att_dst2, b2, ln_w, ln_b, res_W, res_b):
    N, C1 = x.shape
    R = resource_features.shape[1]
    res_dim = resource_features.shape[2]
    heads = att_src1.shape[0]
    hid = W2.shape[1]
    rhs1, ones1, rhs2, ones2 = pack_weights(
        W1, att_src1, att_dst1, W2, att_src2, att_dst2, hid, heads)

    x_pad = np.zeros((g.node_pad, C1), dtype=np.float32)
    x_pad[:N] = x[g.order]
    xT_pad = np.ascontiguousarray(x_pad.T).astype(nbf)
    consts = np.zeros((8, 512), dtype=np.float32)
    consts[0, 0:2 * hid] = b1
    consts[1, 0:hid] = b2
    consts[2, 0:hid] = ln_w
    consts[3, 0:hid] = ln_b
    consts[7, 0:P] = 1.0
    ident = np.eye(P, dtype=np.float32).astype(nbf)
    resw65 = np.zeros((res_dim + 1, hid), dtype=np.float32)
    resw65[0:res_dim] = res_W
    resw65[res_dim] = res_b

    res_perm = resource_features[g.order].reshape(N * R, res_dim)
    RROW = g.npc * R

    common = {
        "xT_bf": xT_pad,
        "rhs1": rhs1.astype(nbf), "ones1row": ones1.astype(nbf),
        "rhs2": rhs2.astype(nbf), "ones2row": ones2.astype(nbf),
        "resw": resw65.astype(nbf),
        "consts": consts, "ident": ident,
    }
    in_maps = []
    for k in range(g.n_cores):
        # core k owns new-ids n with (n//P) % n_cores == k, in (j, p) order
        jj = np.arange(g.npc)
        n_ids = ((jj // P) * g.n_cores + k) * P + jj % P
        valid = n_ids < N
        rc = np.zeros((RROW, res_dim), dtype=np.float32)
        rows = np.repeat(jj[valid], R) * R + np.tile(np.arange(R),
                                                     valid.sum())
        src_rows = np.repeat(n_ids[valid], R) * R + np.tile(
            np.arange(R), valid.sum())
        rc[rows] = res_perm[src_rows]
        in_maps.append(dict(
            common,
            resT_bf=np.ascontiguousarray(rc.T).astype(nbf),
            ix1=g.ix1[k], ix2=g.ix2[k], jm=g.jm[k], ownmask=g.ownmask[k],
        ))
    return in_maps


def _install_ntff_hook():
    import sys, types, contextlib, ctypes
    if "antenv.axon_hooks" in sys.modules:
        return
    so_path = "/opt/axon/libaxon_pjrt.so"
    mod = types.ModuleType("antenv.axon_hooks")
    _h = [None]
    mod.set_axon_ntff_profile_hook = lambda h: _h.__setitem__(0, h)
    mod.get_axon_ntff_profile_hook = lambda: _h[0]
    sys.modules["antenv.axon_hooks"] = mod
    try:
        lib = ctypes.CDLL(so_path)
        if not hasattr(lib, "axon_start_nrt_profile"):
            return
        lib.axon_start_nrt_profile.argtypes = [
            ctypes.POINTER(ctypes.c_int64), ctypes.c_size_t]
        lib.axon_start_nrt_profile.restype = ctypes.c_int64
        lib.axon_stop_nrt_profile.argtypes = [ctypes.c_char_p]
        lib.axon_stop_nrt_profile.restype = ctypes.c_int64

        @contextlib.contextmanager
        def _hook(output_dir, device_ids):
            import jax
            jax.devices()
            if device_ids:
                ids = (ctypes.c_int64 * len(device_ids))(*device_ids)
                rc = lib.axon_start_nrt_profile(ids, len(device_ids))
            else:
                rc = lib.axon_start_nrt_profile(None, 0)
            if rc != 0:
                raise RuntimeError(f"axon_start_nrt_profile rc={rc}")
            try:
                yield
            finally:
                n = lib.axon_stop_nrt_profile(str(output_dir).encode())
                print(f"ntff profile: {n} file(s) -> {output_dir}")

        mod.set_axon_ntff_profile_hook(_hook)
    except Exception as e:
        print("ntff hook install failed:", e)


_CACHE = {}


def kernel(x, edge_index, resource_features, W1, att_src1, att_dst1, b1,
           W2, att_src2, att_dst2, b2, ln_w, ln_b, res_W, res_b, *,
           n_cores=8, _trace=False):
    from concourse.bass_utils import run_bass_kernel_spmd
    if _trace:
        _install_ntff_hook()

    x = np.asarray(x, np.float32)
    edge_index = np.asarray(edge_index)
    resource_features = np.asarray(resource_features, np.float32)
    N, C1 = x.shape
    R = resource_features.shape[1]
    res_dim = resource_features.shape[2]
    att_src1 = np.asarray(att_src1, np.float32)
    heads = att_src1.shape[0]
    W2 = np.asarray(W2, np.float32)
    hid = W2.shape[1]

    key = ("prog", N, edge_index.shape[1])
    if key in _CACHE:
        g, nc = _CACHE[key]
    else:
        g = build_geometry(N, n_cores, edge_index[0], edge_index[1])
        nc = build_program(g, hid=hid, heads=heads, C1=C1, R=R,
                           res_dim=res_dim)
        _CACHE[key] = (g, nc)

    in_maps = make_inputs(
        g, x, resource_features, np.asarray(W1, np.float32), att_src1,
        np.asarray(att_dst1, np.float32), np.asarray(b1, np.float32),
        W2, np.asarray(att_src2, np.float32), np.asarray(att_dst2, np.float32),
        np.asarray(b2, np.float32), np.asarray(ln_w, np.float32),
        np.asarray(ln_b, np.float32), np.asarray(res_W, np.float32),
        np.asarray(res_b, np.float32))

    res = run_bass_kernel_spmd(nc, in_maps, list(range(n_cores)),
                               trace=_trace)
    outs = [np.asarray(res.results[k]["out"]) for k in range(n_cores)]
    full = np.zeros((N, R, 2 * hid), dtype=np.float32)
    for k in range(n_cores):
        jj = np.arange(g.npc)
        n_ids = ((jj // P) * n_cores + k) * P + jj % P
        valid = n_ids < N
        full[g.order[n_ids[valid]]] = outs[k][valid]
    if _trace:
        kernel.last_exec_time_ns = res.exec_time_ns
    return full.astype(np.float32)


# revision 25
# speedup vs baseline: 1.1569x; 1.1569x over previous
"""CloudResourceGNN (2-layer GAT + resource embedding) on 8 Trainium2 NeuronCores.

Layout: nodes sorted by in-degree (desc) and dealt into 128-node blocks;
global block g -> core g%8, slot g//8, partition = n%128. Edges grouped by
dst block; slot (tile i, partition p) holds the i-th edge (src-sorted) of
dst node p in the block, padded to the block's max degree. With partition ==
dst: a_dst is a per-partition Activation-engine bias, the softmax scatter is
an identity-lhsT PSUM accumulate, and no per-edge dst-side gathers exist.
Src rows are fetched with one SWDGE dma_gather per (layer, block) from
pair-packed tables (1024B rows L1 / 512B L2, idx = src>>1, int16-safe);
parity is resolved by dual masked matmuls. Softmax runs without
max-subtraction: w = exp(leaky_relu(asrc+adst)) with denominators from ones
columns. LayerNorm runs on the Activation engine via ln/exp (one act table).
The graph-independent resource-embedding half of the output is computed and
written during the layer-1 edge phase.
"""

import numpy as np
import ml_dtypes

import concourse.bass as bass
import concourse.bacc as bacc
import concourse.mybir as mybir
import concourse.tile as tile

BF16 = mybir.dt.bfloat16
F32 = mybir.dt.float32
I16 = mybir.dt.int16
OPc = mybir.AluOpType
AF = mybir.ActivationFunctionType
nbf = ml_dtypes.bfloat16

NEG_SLOPE = 0.2
LN_EPS = 1e-5
P = 128


class Geo:
    pass


def _wrap16(vals):
    """idx list (len % 128 == 0) -> [128, n/16] wrapped-16, replicated x8."""
    v = np.asarray(vals, np.int64)
    assert len(v) % 128 == 0
    w = v.reshape(-1, 16).T                    # [16, n/16]
    return np.tile(w, (8, 1)).astype(np.int16)  # [128, n/16]


def build_geometry(N, n_cores, src, dst, cap=20):
    g = Geo()
    g.N = N
    g.n_cores = n_cores
    per_core_nodes = -(-N // n_cores)
    g.nblk = -(-per_core_nodes // P)
    g.npc = g.nblk * P
    g.node_pad = g.npc * n_cores
    g.ntile = g.node_pad // P          # global blocks

    loop = np.arange(N, dtype=np.int64)
    s_all = np.concatenate([np.asarray(src, np.int64), loop])
    d_all = np.concatenate([np.asarray(dst, np.int64), loop])
    deg = np.bincount(d_all, minlength=N)

    # degree-sorted placement: new id n = rank in descending-degree order
    g.order = np.argsort(-deg, kind="stable")          # new -> old
    pos = np.empty(N, np.int64)
    pos[g.order] = np.arange(N)                        # old -> new
    g.pos = pos

    sn = pos[s_all]
    dn = pos[d_all]
    gblk = dn // P                                     # global dst block
    core = gblk % n_cores
    lblk = gblk // n_cores                             # local block slot
    part = dn % P

    # tiles per local-block slot j: max over cores/partitions of per-node deg
    degn = np.zeros(g.node_pad, np.int64)
    degn[:N] = deg[g.order]
    dmax = degn.reshape(g.ntile, P).max(axis=1)        # per global block
    g.Tb = np.zeros(g.nblk, np.int64)
    for j in range(g.nblk):
        g.Tb[j] = max(1, dmax[j * n_cores:(j + 1) * n_cores].max())
    g.T = int(g.Tb.sum())
    g.S = g.T * P
    g.t0 = np.zeros(g.nblk, np.int64)
    g.t0[1:] = np.cumsum(g.Tb)[:-1]
    g.gmax = min(cap, int(g.Tb.max()))
    # chunks: (block, tile_lo, tile_hi, first, last) with tile_hi-tile_lo <= cap
    g.chunks = []
    for j in range(g.nblk):
        lo = 0
        while lo < g.Tb[j]:
            hi = min(lo + cap, int(g.Tb[j]))
            g.chunks.append((j, lo, hi, lo == 0, hi == g.Tb[j]))
            lo = hi

    # layer-2 global row of node n (AllGather order: core-major)
    def t2row(n):
        gb = n // P
        return (gb % n_cores) * g.npc + (gb // n_cores) * P + n % P

    g.ix1 = np.zeros((n_cores, P, g.S // 16), np.int16)
    g.ix2 = np.zeros((n_cores, P, g.S // 16), np.int16)
    g.jm = np.zeros((n_cores, P, 2 * g.T), np.float32)
    g.ownmask = np.zeros((n_cores, P, g.ntile), np.float32)

    for k in range(n_cores):
        m = core == k
        s, j, p = sn[m], lblk[m], part[m]
        # sort edges by (block, partition, src) so slot i of node p is its
        # i-th smallest src (quantile locality across partitions)
        o = np.lexsort((s, p, j))
        s, j, p = s[o], j[o], p[o]
        v1 = np.zeros(g.S, np.int64)
        v2 = np.zeros(g.S, np.int64)
        pme = np.zeros(g.S, np.float32)
        pmo = np.zeros(g.S, np.float32)
        vld = np.zeros(g.S, bool)
        # slot index: i-th edge of (j, p) -> (g.t0[j] + i) * P + p
        # compute i via cumcount within (j, p) runs
        if len(s):
            key = j * P + p
            start = np.r_[0, np.nonzero(np.diff(key))[0] + 1]
            runlen = np.diff(np.r_[start, len(key)])
            within = np.arange(len(key)) - np.repeat(start, runlen)
            slot = (g.t0[j] + within) * P + p
            v1[slot] = s >> 1
            v2[slot] = t2row(s) >> 1
            vld[slot] = True
            even = (s % 2 == 0).astype(np.float32)
            pme[slot] = even
            pmo[slot] = 1.0 - even
        # trailing-pad slots per chunk -> idx -1 (Q7 trims them)
        for (j2, tl, th, _, _) in g.chunks:
            a, b = (g.t0[j2] + tl) * P, (g.t0[j2] + th) * P
            e = b
            while e > a and not vld[e - 1]:
                e -= 1
            g.ix1[k, :, a // 16:b // 16] = _wrap16(v1[a:b])
            g.ix2[k, :, a // 16:b // 16] = _wrap16(v2[a:b])
        jm = np.stack([pme, pmo], axis=-1).reshape(g.T, P, 2)
        g.jm[k] = jm.transpose(1, 0, 2).reshape(P, 2 * g.T)
        own = np.zeros(g.ntile, np.float32)
        own[np.arange(g.ntile) % n_cores == k] = 1.0
        g.ownmask[k] = np.tile(own, (P, 1))
    return g


def pack_weights(W1, att_src1, att_dst1, W2, att_src2, att_dst2, hid, heads):
    C1 = W1.shape[0]
    n1 = 2 * (hid + 1) + 2 * heads      # 134: [h0|1|h1|1|as0,as1|ad0,ad1]
    rhs1 = np.zeros((C1, n1), dtype=np.float32)
    rhs1[:, 0:hid] = W1[:, 0:hid]
    rhs1[:, hid + 1:2 * hid + 1] = W1[:, hid:2 * hid]
    Wh = W1.reshape(C1, heads, hid)
    rhs1[:, 2 * hid + 2:2 * hid + 2 + heads] = np.einsum("ihc,hc->ih", Wh, att_src1)
    rhs1[:, 2 * hid + 2 + heads:] = np.einsum("ihc,hc->ih", Wh, att_dst1)
    ones1 = np.zeros((1, n1), dtype=np.float32)
    ones1[0, hid] = 1.0
    ones1[0, 2 * hid + 1] = 1.0
    C2 = W2.shape[0]
    n2 = hid + 3                        # 67: [h|1|asrc2|adst2]
    rhs2 = np.zeros((C2, n2), dtype=np.float32)
    rhs2[:, 0:hid] = W2
    rhs2[:, hid + 1] = W2 @ att_src2[0]
    rhs2[:, hid + 2] = W2 @ att_dst2[0]
    ones2 = np.zeros((1, n2), dtype=np.float32)
    ones2[0, hid] = 1.0
    return rhs1, ones1, rhs2, ones2


def build_program(g, hid=64, heads=2, C1=128, R=16, res_dim=64):
    NT = g.ntile
    NB = g.nblk
    n1 = 2 * (hid + 1) + 2 * heads      # 134
    n2 = hid + 3                        # 67
    w1c = hid + 1                       # 65
    T1C = 256                           # table1 cols per node (bf16, 512B)
    T2C = 128                           # table2 cols per node (bf16, 256B)
    RROW = NB * P * R

    nc = bacc.Bacc("TRN2", target_bir_lowering=False, debug=False,
                   num_devices=g.n_cores)

    xT_bf = nc.dram_tensor("xT_bf", [C1, g.node_pad], BF16, kind="ExternalInput")
    rhs1_d = nc.dram_tensor("rhs1", [C1, n1], BF16, kind="ExternalInput")
    ones1_d = nc.dram_tensor("ones1row", [1, n1], BF16, kind="ExternalInput")
    rhs2_d = nc.dram_tensor("rhs2", [C1, n2], BF16, kind="ExternalInput")
    ones2_d = nc.dram_tensor("ones2row", [1, n2], BF16, kind="ExternalInput")
    resw_d = nc.dram_tensor("resw", [res_dim + 1, hid], BF16, kind="ExternalInput")
    consts_d = nc.dram_tensor("consts", [8, 512], F32, kind="ExternalInput")
    ident_d = nc.dram_tensor("ident", [P, P], BF16, kind="ExternalInput")
    resT_d = nc.dram_tensor("resT_bf", [res_dim, RROW], BF16, kind="ExternalInput")
    ix1_d = nc.dram_tensor("ix1", [P, g.S // 16], I16, kind="ExternalInput")
    ix2_d = nc.dram_tensor("ix2", [P, g.S // 16], I16, kind="ExternalInput")
    jm_d = nc.dram_tensor("jm", [P, 2 * g.T], F32, kind="ExternalInput")
    own_d = nc.dram_tensor("ownmask", [P, NT], F32, kind="ExternalInput")
    out_d = nc.dram_tensor("out", [g.npc, R, 2 * hid], F32, kind="ExternalOutput")

    table1 = nc.dram_tensor("table1", [g.node_pad, T1C], BF16)
    myblk2 = nc.dram_tensor("myblk2", [g.npc, T2C], BF16)
    table2 = nc.dram_tensor("table2", [g.n_cores * g.npc, T2C], BF16,
                            addr_space="Shared")
    t1pair = table1.ap().rearrange("(r x) c -> r (x c)", x=2)   # [np/2, 512]
    t2pair = table2.ap().rearrange("(r x) c -> r (x c)", x=2)   # [np*8/2? -> 512]

    with tile.TileContext(nc) as tc:
        with tc.tile_pool(name="consts", bufs=1) as cpool, \
             tc.tile_pool(name="persist", bufs=1) as pp:
            crow = []
            for r in range(8):
                t_ = cpool.tile([1, 512], F32, tag=f"crow{r}", name=f"crow{r}")
                nc.sync.dma_start(out=t_[:, :], in_=consts_d[r:r + 1, :])
                crow.append(t_)
            onesbf = cpool.tile([1, P], BF16)
            nc.vector.tensor_copy(out=onesbf[:, :], in_=crow[7][:, 0:P])
            eps_t = cpool.tile([P, 1], F32)
            nc.vector.memset(eps_t[:, :], LN_EPS)
            grd_t = cpool.tile([P, 1], F32)
            nc.vector.memset(grd_t[:, :], 1e-20)
            ident_s = cpool.tile([P, P], BF16)
            nc.sync.dma_start(out=ident_s[:, :], in_=ident_d[:, :])
            rhs1_s = cpool.tile([C1, n1], BF16)
            nc.sync.dma_start(out=rhs1_s[:, :], in_=rhs1_d[:, :])
            ones1_s = cpool.tile([1, n1], BF16)
            nc.sync.dma_start(out=ones1_s[:, :], in_=ones1_d[:, :])
            rhs2_s = cpool.tile([C1, n2], BF16)
            nc.sync.dma_start(out=rhs2_s[:, :], in_=rhs2_d[:, :])
            ones2_s = cpool.tile([1, n2], BF16)
            nc.sync.dma_start(out=ones2_s[:, :], in_=ones2_d[:, :])
            resw_s = cpool.tile([res_dim + 1, hid], BF16)
            nc.sync.dma_start(out=resw_s[:, :], in_=resw_d[:, :])

            ones_f = cpool.tile([1, P], F32)
            nc.vector.tensor_copy(out=ones_f[:, :], in_=crow[7][:, 0:P])
            b1_rep = cpool.tile([P, 2 * hid], F32)
            b2_rep = cpool.tile([P, hid], F32)
            lnw_rep = cpool.tile([P, hid], F32)
            lnb_rep = cpool.tile([P, hid], F32)
            with tc.tile_pool(name="repl_ps", bufs=2, space="PSUM") as rps:
                for dst_t, row, ncol in (
                    (b1_rep, 0, 2 * hid), (b2_rep, 1, hid),
                    (lnw_rep, 2, hid), (lnb_rep, 3, hid),
                ):
                    pst = rps.tile([P, 512], F32, tag="repl", name=f"repl{row}")
                    nc.tensor.matmul(out=pst[:, 0:ncol], lhsT=ones_f[:, :],
                                     rhs=crow[row][:, 0:ncol],
                                     start=True, stop=True)
                    nc.vector.tensor_copy(out=dst_t[:, 0:ncol],
                                          in_=pst[:, 0:ncol])

            jm_sb = pp.tile([P, g.T, 2], F32)
            nc.sync.dma_start(out=jm_sb[:, :, :], in_=jm_d[:, :])
            ix1_sb = pp.tile([P, g.S // 16], I16)
            nc.sync.dma_start(out=ix1_sb[:, :], in_=ix1_d[:, :])
            ix2_sb = pp.tile([P, g.S // 16], I16)
            nc.sync.dma_start(out=ix2_sb[:, :], in_=ix2_d[:, :])
            ownm = pp.tile([P, NT], F32)
            nc.sync.dma_start(out=ownm[:, :], in_=own_d[:, :])
            blk2_sb = pp.tile([P, NB, T2C], BF16)
            adst1_sb = pp.tile([P, NB, heads], F32)
            adst2_sb = pp.tile([P, NB, 1], F32)
            nc.vector.memset(adst1_sb[:, :, :], 0.0)
            nc.vector.memset(adst2_sb[:, :, :], 0.0)
            nc.vector.memset(blk2_sb[:, :, :], 0.0)

            # ---------------- phase 1: node phase (replicated) -------------
            XCH = 32
            with tc.tile_pool(name="n1_xt", bufs=2) as xtp, \
                 tc.tile_pool(name="n1_ps", bufs=4, space="PSUM") as n1ps, \
                 tc.tile_pool(name="n1_st", bufs=3) as n1st:
                nch = -(-NT // XCH)
                sb_iter = 0
                for c in range(nch):
                    tn0 = c * XCH
                    ntl = min(XCH, NT - tn0)
                    xt = xtp.tile([P, XCH * P], BF16, tag="xt")
                    nc.sync.dma_start(out=xt[:, 0:ntl * P],
                                      in_=xT_bf[:, tn0 * P:(tn0 + ntl) * P])
                    nst = -(-ntl // 4)
                    for sb in range(nst):
                        st = n1st.tile([P, 4, T1C], BF16, tag="n1st")
                        nn = min(4, ntl - sb * 4)
                        if sb_iter < 3:
                            nc.vector.memset(st[:, :, n1 - 2:T1C], 0.0)
                        sb_iter += 1
                        for i in range(nn):
                            t = sb * 4 + i
                            gt_ = tn0 + t
                            b = gt_ // g.n_cores
                            ps = n1ps.tile([P, n1], F32, tag="n1ps")
                            nc.tensor.matmul(out=ps[:, :],
                                             lhsT=xt[:, t * P:(t + 1) * P],
                                             rhs=rhs1_s[:, :],
                                             start=True, stop=False)
                            nc.tensor.matmul(out=ps[:, :], lhsT=onesbf[:, :],
                                             rhs=ones1_s[:, :],
                                             start=False, stop=True)
                            nc.scalar.copy(out=st[:, i:i + 1, 0:n1 - 2],
                                           in_=ps[:, 0:n1 - 2])
                            nc.vector.scalar_tensor_tensor(
                                out=adst1_sb[:, b:b + 1, 0:heads],
                                in0=ps[:, n1 - 2:n1],
                                scalar=ownm[:, gt_:gt_ + 1],
                                in1=adst1_sb[:, b:b + 1, 0:heads],
                                op0=OPc.mult, op1=OPc.add)
                        nc.sync.dma_start(
                            out=table1.ap().rearrange(
                                "(t p) c -> p t c",
                                p=P)[:, tn0 + sb * 4:tn0 + sb * 4 + nn, :],
                            in_=st[:, 0:nn, :])

            # ------- phase 2: layer-1 edge phase + res embedding -----------
            with tc.tile_pool(name="e1_g", bufs=3) as gp, \
                 tc.tile_pool(name="e1_w", bufs=2) as wp, \
                 tc.tile_pool(name="e1_ps", bufs=2, space="PSUM") as eps, \
                 tc.tile_pool(name="e1_tp", bufs=1, space="PSUM") as tps, \
                 tc.tile_pool(name="e1_h2", bufs=1, space="PSUM") as h2ps, \
                 tc.tile_pool(name="e1_x2", bufs=2) as x2p, \
                 tc.tile_pool(name="res_t", bufs=2) as resp, \
                 tc.tile_pool(name="res_ps", bufs=2, space="PSUM") as rps2:
                res_iter = 0
                g_iter = 0
                psum_cur = {}
                for ci, (b, tl, th, first, last) in enumerate(g.chunks):
                    t0 = int(g.t0[b]) + tl
                    ntl = th - tl
                    gt = gp.tile([P, g.gmax, 2 * T1C], BF16, tag="g1",
                                 name=f"g1_{ci}")
                    if g_iter < 3:
                        nc.vector.memset(gt[:, :, :], 0.0)
                    g_iter += 1
                    nc.gpsimd.dma_gather(
                        gt[:, 0:ntl, :], t1pair,
                        ix1_sb[:, t0 * 8:(t0 + ntl) * 8], ntl * P, ntl * P,
                        2 * T1C, single_packet=False)
                    # w = exp(leaky(asrc + adst)) per (parity, head)
                    wt = wp.tile([P, g.gmax, 4], F32, tag="w1", name=f"w1_{ci}")
                    for par in range(2):
                        for h in range(heads):
                            c = par * heads + h
                            ac = par * T1C + n1 - 4 + h
                            nc.scalar.activation(
                                out=wt[:, 0:ntl, c:c + 1],
                                in_=gt[:, 0:ntl, ac:ac + 1],
                                func=AF.Identity,
                                bias=adst1_sb[:, b, h:h + 1])
                    nc.vector.scalar_tensor_tensor(
                        out=wt[:, 0:ntl, :], in0=wt[:, 0:ntl, :],
                        scalar=NEG_SLOPE, in1=wt[:, 0:ntl, :],
                        op0=OPc.mult, op1=OPc.max)
                    nc.scalar.activation(out=wt[:, 0:ntl, :],
                                         in_=wt[:, 0:ntl, :], func=AF.Exp)
                    nc.vector.tensor_tensor(
                        out=wt[:, 0:ntl, 0:2], in0=wt[:, 0:ntl, 0:2],
                        in1=jm_sb[:, t0:t0 + ntl, 0:1].to_broadcast(
                            [P, ntl, 2]), op=OPc.mult)
                    nc.vector.tensor_tensor(
                        out=wt[:, 0:ntl, 2:4], in0=wt[:, 0:ntl, 2:4],
                        in1=jm_sb[:, t0:t0 + ntl, 1:2].to_broadcast(
                            [P, ntl, 2]), op=OPc.mult)
                    gs = gp.tile([P, g.gmax, 4 * w1c], BF16, tag="gs",
                                 name=f"gs_{ci}")
                    for par in range(2):
                        for h in range(heads):
                            nc.vector.tensor_tensor(
                                out=gs[:, 0:ntl,
                                       (par * heads + h) * w1c:
                                       (par * heads + h + 1) * w1c],
                                in0=gt[:, 0:ntl,
                                       par * T1C + h * w1c:
                                       par * T1C + (h + 1) * w1c],
                                in1=wt[:, 0:ntl,
                                       par * heads + h:par * heads + h + 1
                                       ].to_broadcast([P, ntl, w1c]),
                                op=OPc.mult)
                    if first:
                        psum_cur[b] = eps.tile([P, heads * w1c], F32,
                                               tag="e1ps", name=f"e1ps_{b}")
                    pc = psum_cur[b]
                    for i in range(ntl):
                        for par in range(2):
                            nc.tensor.matmul(
                                out=pc[:, :], lhsT=ident_s[:, :],
                                rhs=gs[:, i:i + 1,
                                       par * heads * w1c:
                                       (par + 1) * heads * w1c],
                                start=(first and i == 0 and par == 0),
                                stop=(last and i == ntl - 1 and par == 1))
                    if not last:
                        continue
                    pc = psum_cur.pop(b)
                    # epilogue: softmax div + bias + ELU -> x2t
                    x2pre = x2p.tile([P, 2 * hid], F32, tag="x2pre",
                                     name=f"x2pre_{b}")
                    esc = x2p.tile([P, 2 * hid], F32, tag="esc",
                                   name=f"esc_{b}")
                    x2t = x2p.tile([P, 2 * hid], BF16, tag="x2",
                                   name=f"x2_{b}")
                    for h in range(heads):
                        rec = x2p.tile([P, 1], F32, tag=f"rec{h}",
                                       name=f"rec{h}_{b}")
                        dn = x2p.tile([P, 1], F32, tag=f"dn{h}",
                                      name=f"dn{h}_{b}")
                        nc.vector.tensor_scalar(
                            out=dn[:, :], in0=pc[:, (h + 1) * w1c - 1:
                                                 (h + 1) * w1c],
                            scalar1=1e-30, scalar2=None, op0=OPc.add)
                        nc.vector.reciprocal(out=rec[:, :], in_=dn[:, :])
                        nc.vector.scalar_tensor_tensor(
                            out=x2pre[:, h * hid:(h + 1) * hid],
                            in0=pc[:, h * w1c:h * w1c + hid],
                            scalar=rec[:, 0:1],
                            in1=b1_rep[:, h * hid:(h + 1) * hid],
                            op0=OPc.mult, op1=OPc.add)
                    # ELU: x2t = max(exp(min(x,0)) - 1, x)
                    nc.scalar.activation(out=esc[:, :], in_=x2pre[:, :],
                                         func=AF.Relu, scale=-1.0)
                    nc.scalar.activation(out=esc[:, :], in_=esc[:, :],
                                         func=AF.Exp, scale=-1.0)
                    nc.vector.scalar_tensor_tensor(
                        out=x2t[:, :], in0=esc[:, :], scalar=-1.0,
                        in1=x2pre[:, :], op0=OPc.add, op1=OPc.max)
                    tp = tps.tile([P, P], BF16, tag="x2tp", name=f"tp_{b}")
                    nc.tensor.transpose(out=tp[:, :], in_=x2t[:, :],
                                        identity=ident_s[:, :])
                    x2tt = x2p.tile([P, P], BF16, tag="x2tt", name=f"x2tt_{b}")
                    nc.scalar.copy(out=x2tt[:, :], in_=tp[:, :])
                    h2 = h2ps.tile([P, n2], F32, tag="h2ps", name=f"h2_{b}")
                    nc.tensor.matmul(out=h2[:, :], lhsT=x2tt[:, :],
                                     rhs=rhs2_s[:, :], start=True, stop=False)
                    nc.tensor.matmul(out=h2[:, :], lhsT=onesbf[:, :],
                                     rhs=ones2_s[:, :], start=False, stop=True)
                    nc.scalar.copy(out=blk2_sb[:, b:b + 1, 0:n2 - 1],
                                   in_=h2[:, 0:n2 - 1])
                    nc.vector.tensor_copy(out=adst2_sb[:, b:b + 1, 0:1],
                                          in_=h2[:, n2 - 1:n2])

                    # res embedding for this block (graph-independent)
                    rt = resp.tile([res_dim + 1, P, R], BF16, tag="rest",
                                   name=f"rt_{b}")
                    if res_iter < 2:
                        nc.vector.memset(rt[res_dim:res_dim + 1, :, :], 1.0)
                    res_iter += 1
                    nc.sync.dma_start(
                        out=rt[0:res_dim, :, :],
                        in_=resT_d[:, b * P * R:(b + 1) * P * R])
                    for half in range(2):
                        rp = rps2.tile([P, 8 * hid], F32, tag="resps",
                                       name=f"rp_{b}_{half}")
                        for r8 in range(8):
                            r = half * 8 + r8
                            nc.tensor.matmul(
                                out=rp[:, r8 * hid:(r8 + 1) * hid],
                                lhsT=rt[:, :, r:r + 1],
                                rhs=resw_s[:, :], start=True, stop=True)
                        em = resp.tile([P, 8, hid], F32, tag="em",
                                       name=f"em_{b}_{half}")
                        ro = resp.tile([P, 8, hid], F32, tag="ro",
                                       name=f"ro_{b}_{half}")
                        nc.scalar.activation(out=em[:, :, :], in_=rp[:, :],
                                             func=AF.Relu, scale=-1.0)
                        nc.scalar.activation(out=em[:, :, :], in_=em[:, :, :],
                                             func=AF.Exp, scale=-1.0)
                        nc.vector.scalar_tensor_tensor(
                            out=ro[:, :, :], in0=em[:, :, :], scalar=-1.0,
                            in1=rp[:, :], op0=OPc.add, op1=OPc.max)
                        nc.sync.dma_start(
                            out=out_d[b * P:(b + 1) * P,
                                      half * 8:(half + 1) * 8, hid:2 * hid],
                            in_=ro[:, :, :])

            nc.sync.dma_start(
                out=myblk2.ap().rearrange("(j p) c -> p j c", p=P)[:, :, :],
                in_=blk2_sb[:, :, :])
            nc.gpsimd.collective_compute(
                "AllGather", OPc.bypass,
                replica_groups=[list(range(g.n_cores))],
                ins=[myblk2.ap().opt()],
                outs=[table2.ap().opt()],
            )

            # -------- phase 3: layer-2 edge phase + LN + output ------------
            GRP = 8
            with tc.tile_pool(name="e2_g", bufs=3) as gp2, \
                 tc.tile_pool(name="e2_w", bufs=2) as wp2, \
                 tc.tile_pool(name="e2_ps", bufs=2, space="PSUM") as eps2, \
                 tc.tile_pool(name="ln", bufs=2) as lnp, \
                 tc.tile_pool(name="lng", bufs=2) as lgp:
                g2_iter = 0
                psum2 = {}
                xcg = None
                for ci, (b, tl, th, first, last) in enumerate(g.chunks):
                    t0 = int(g.t0[b]) + tl
                    ntl = th - tl
                    gt2 = gp2.tile([P, g.gmax, 2 * T2C], BF16, tag="g2",
                                   name=f"g2_{ci}")
                    if g2_iter < 3:
                        nc.vector.memset(gt2[:, :, :], 0.0)
                    g2_iter += 1
                    nc.gpsimd.dma_gather(
                        gt2[:, 0:ntl, :], t2pair,
                        ix2_sb[:, t0 * 8:(t0 + ntl) * 8], ntl * P, ntl * P,
                        2 * T2C, single_packet=False)
                    wt2 = wp2.tile([P, g.gmax, 2], F32, tag="w2",
                                   name=f"w2_{ci}")
                    for par in range(2):
                        ac = par * T2C + hid + 1
                        nc.scalar.activation(
                            out=wt2[:, 0:ntl, par:par + 1],
                            in_=gt2[:, 0:ntl, ac:ac + 1],
                            func=AF.Identity,
                            bias=adst2_sb[:, b, 0:1])
                    nc.vector.scalar_tensor_tensor(
                        out=wt2[:, 0:ntl, :], in0=wt2[:, 0:ntl, :],
                        scalar=NEG_SLOPE, in1=wt2[:, 0:ntl, :],
                        op0=OPc.mult, op1=OPc.max)
                    nc.scalar.activation(out=wt2[:, 0:ntl, :],
                                         in_=wt2[:, 0:ntl, :], func=AF.Exp)
                    nc.vector.tensor_tensor(
                        out=wt2[:, 0:ntl, :], in0=wt2[:, 0:ntl, :],
                        in1=jm_sb[:, t0:t0 + ntl, :], op=OPc.mult)
                    gs2 = gp2.tile([P, g.gmax, 2 * w1c], BF16, tag="gs2",
                                   name=f"gs2_{ci}")
                    for par in range(2):
                        nc.vector.tensor_tensor(
                            out=gs2[:, 0:ntl, par * w1c:(par + 1) * w1c],
                            in0=gt2[:, 0:ntl, par * T2C:par * T2C + w1c],
                            in1=wt2[:, 0:ntl, par:par + 1].to_broadcast(
                                [P, ntl, w1c]),
                            op=OPc.mult)
                    if first:
                        psum2[b] = eps2.tile([P, w1c], F32, tag="e2ps",
                                             name=f"e2ps_{b}")
                    ps2 = psum2[b]
                    for i in range(ntl):
                        for par in range(2):
                            nc.tensor.matmul(
                                out=ps2[:, :], lhsT=ident_s[:, :],
                                rhs=gs2[:, i:i + 1,
                                        par * w1c:(par + 1) * w1c],
                                start=(first and i == 0 and par == 0),
                                stop=(last and i == ntl - 1 and par == 1))
                    if not last:
                        continue
                    ps2 = psum2.pop(b)
                    jg = b % GRP
                    if jg == 0:
                        xcg = lgp.tile([P, GRP, hid], F32, tag="xcg",
                                       name=f"xcg_{b}")
                        mvg = lgp.tile([P, GRP, 2], F32, tag="mvg",
                                       name=f"mvg_{b}")
                        sdg = lgp.tile([P, GRP], F32, tag="sdg",
                                       name=f"sdg_{b}")
                        rsg = lgp.tile([P, GRP], F32, tag="rsg",
                                       name=f"rsg_{b}")
                    # softmax div + bias -> y; mean/var on DVE (bn_stats)
                    y = lnp.tile([P, hid], F32, tag="y", name=f"y_{b}")
                    rec = lnp.tile([P, 1], F32, tag="rec2", name=f"r2_{b}")
                    dn2 = lnp.tile([P, 1], F32, tag="dn2", name=f"d2_{b}")
                    st6 = lnp.tile([P, 6], F32, tag="st6", name=f"s6_{b}")
                    nc.vector.tensor_scalar(
                        out=dn2[:, :], in0=ps2[:, hid:hid + 1],
                        scalar1=1e-30, scalar2=None, op0=OPc.add)
                    nc.vector.reciprocal(out=rec[:, :], in_=dn2[:, :])
                    nc.vector.scalar_tensor_tensor(
                        out=y[:, :], in0=ps2[:, 0:hid], scalar=rec[:, 0:1],
                        in1=b2_rep[:, :], op0=OPc.mult, op1=OPc.add)
                    nc.vector.bn_stats(out=st6[:, :], in_=y[:, :])
                    nc.vector.bn_aggr(out=mvg[:, jg, :], in_=st6[:, :])
                    nc.vector.tensor_scalar(
                        out=xcg[:, jg, :], in0=y[:, :],
                        scalar1=mvg[:, jg, 0:1], scalar2=None,
                        op0=OPc.subtract)
                    if jg == GRP - 1 or b == NB - 1:
                        gn = jg + 1
                        b0 = b - jg
                        nc.scalar.activation(out=sdg[:, 0:gn],
                                             in_=mvg[:, 0:gn, 1],
                                             func=AF.Sqrt,
                                             bias=eps_t[:, 0:1])
                        nc.vector.reciprocal(out=rsg[:, 0:gn],
                                             in_=sdg[:, 0:gn])
                        for j2 in range(gn):
                            bb = b0 + j2
                            lnh = lnp.tile([P, 1, hid], F32, tag="lnh",
                                           name=f"lnh_{bb}")
                            nc.vector.scalar_tensor_tensor(
                                out=lnh[:, 0, :], in0=xcg[:, j2, :],
                                scalar=rsg[:, j2:j2 + 1],
                                in1=lnw_rep[:, :],
                                op0=OPc.mult, op1=OPc.mult)
                            nc.vector.tensor_tensor(out=lnh[:, 0, :],
                                                    in0=lnh[:, 0, :],
                                                    in1=lnb_rep[:, :],
                                                    op=OPc.add)
                            nc.sync.dma_start(
                                out=out_d[bb * P:(bb + 1) * P, :, 0:hid],
                                in_=lnh[:, 0:1, :].to_broadcast(
                                    [P, R, hid]))
    nc.compile()
    return nc


# ----------------------------------------------------------------------------
# host wrapper
# ----------------------------------------------------------------------------

def make_inputs(g, x, resource_features, W1, att_src1, att_dst1, b1,
                W2, att_src2, att_dst2, b2, ln_w, ln_b, res_W, res_b):
    N, C1 = x.shape
    R = resource_features.shape[1]
    res_dim = resource_features.shape[2]
    heads = att_src1.shape[0]
    hid = W2.shape[1]
    rhs1, ones1, rhs2, ones2 = pack_weights(
        W1, att_src1, att_dst1, W2, att_src2, att_dst2, hid, heads)

    x_pad = np.zeros((g.node_pad, C1), dtype=np.float32)
    x_pad[:N] = x[g.order]
    xT_pad = np.ascontiguousarray(x_pad.T).astype(nbf)
    consts = np.zeros((8, 512), dtype=np.float32)
    consts[0, 0:2 * hid] = b1
    consts[1, 0:hid] = b2
    consts[2, 0:hid] = ln_w
    consts[3, 0:hid] = ln_b
    consts[7, 0:P] = 1.0
    ident = np.eye(P, dtype=np.float32).astype(nbf)
    resw65 = np.zeros((res_dim + 1, hid), dtype=np.float32)
    resw65[0:res_dim] = res_W
    resw65[res_dim] = res_b

    res_perm = resource_features[g.order].reshape(N * R, res_dim)
    RROW = g.npc * R

    common = {
        "xT_bf": xT_pad,
        "rhs1": rhs1.astype(nbf), "ones1row": ones1.astype(nbf),
        "rhs2": rhs2.astype(nbf), "ones2row": ones2.astype(nbf),
        "resw": resw65.astype(nbf),
        "consts": consts, "ident": ident,
    }
    in_maps = []
    for k in range(g.n_cores):
        # core k owns new-ids n with (n//P) % n_cores == k, in (j, p) order
        jj = np.arange(g.npc)
        n_ids = ((jj // P) * g.n_cores + k) * P + jj % P
        valid = n_ids < N
        rc = np.zeros((RROW, res_dim), dtype=np.float32)
        rows = np.repeat(jj[valid], R) * R + np.tile(np.arange(R),
                                                     valid.sum())
        src_rows = np.repeat(n_ids[valid], R) * R + np.tile(
            np.arange(R), valid.sum())
        rc[rows] = res_perm[src_rows]
        in_maps.append(dict(
            common,
            resT_bf=np.ascontiguousarray(rc.T).astype(nbf),
            ix1=g.ix1[k], ix2=g.ix2[k], jm=g.jm[k], ownmask=g.ownmask[k],
        ))
    return in_maps


def _install_ntff_hook():
    import sys, types, contextlib, ctypes
    if "antenv.axon_hooks" in sys.modules:
        return
    so_path = "/opt/axon/libaxon_pjrt.so"
    mod = types.ModuleType("antenv.axon_hooks")
    _h = [None]
    mod.set_axon_ntff_profile_hook = lambda h: _h.__setitem__(0, h)
    mod.get_axon_ntff_profile_hook = lambda: _h[0]
    sys.modules["antenv.axon_hooks"] = mod
    try:
        lib = ctypes.CDLL(so_path)
        if not hasattr(lib, "axon_start_nrt_profile"):
            return
        lib.axon_start_nrt_profile.argtypes = [
            ctypes.POINTER(ctypes.c_int64), ctypes.c_size_t]
        lib.axon_start_nrt_profile.restype = ctypes.c_int64
        lib.axon_stop_nrt_profile.argtypes = [ctypes.c_char_p]
        lib.axon_stop_nrt_profile.restype = ctypes.c_int64

        @contextlib.contextmanager
        def _hook(output_dir, device_ids):
            import jax
            jax.devices()
            if device_ids:
                ids = (ctypes.c_int64 * len(device_ids))(*device_ids)
                rc = lib.axon_start_nrt_profile(ids, len(device_ids))
            else:
                rc = lib.axon_start_nrt_profile(None, 0)
            if rc != 0:
                raise RuntimeError(f"axon_start_nrt_profile rc={rc}")
            try:
                yield
            finally:
                n = lib.axon_stop_nrt_profile(str(output_dir).encode())
                print(f"ntff profile: {n} file(s) -> {output_dir}")

        mod.set_axon_ntff_profile_hook(_hook)
    except Exception as e:
        print("ntff hook install failed:", e)


_CACHE = {}


def kernel(x, edge_index, resource_features, W1, att_src1, att_dst1, b1,
           W2, att_src2, att_dst2, b2, ln_w, ln_b, res_W, res_b, *,
           n_cores=8, _trace=False):
    from concourse.bass_utils import run_bass_kernel_spmd
    if _trace:
        _install_ntff_hook()

    x = np.asarray(x, np.float32)
    edge_index = np.asarray(edge_index)
    resource_features = np.asarray(resource_features, np.float32)
    N, C1 = x.shape
    R = resource_features.shape[1]
    res_dim = resource_features.shape[2]
    att_src1 = np.asarray(att_src1, np.float32)
    heads = att_src1.shape[0]
    W2 = np.asarray(W2, np.float32)
    hid = W2.shape[1]

    key = ("prog", N, edge_index.shape[1])
    if key in _CACHE:
        g, nc = _CACHE[key]
    else:
        g = build_geometry(N, n_cores, edge_index[0], edge_index[1])
        nc = build_program(g, hid=hid, heads=heads, C1=C1, R=R,
                           res_dim=res_dim)
        _CACHE[key] = (g, nc)

    in_maps = make_inputs(
        g, x, resource_features, np.asarray(W1, np.float32), att_src1,
        np.asarray(att_dst1, np.float32), np.asarray(b1, np.float32),
        W2, np.asarray(att_src2, np.float32), np.asarray(att_dst2, np.float32),
        np.asarray(b2, np.float32), np.asarray(ln_w, np.float32),
        np.asarray(ln_b, np.float32), np.asarray(res_W, np.float32),
        np.asarray(res_b, np.float32))

    res = run_bass_kernel_spmd(nc, in_maps, list(range(n_cores)),
                               trace=_trace)
    outs = [np.asarray(res.results[k]["out"]) for k in range(n_cores)]
    full = np.zeros((N, R, 2 * hid), dtype=np.float32)
    for k in range(n_cores):
        jj = np.arange(g.npc)
        n_ids = ((jj // P) * n_cores + k) * P + jj % P
        valid = n_ids < N
        full[g.order[n_ids[valid]]] = outs[k][valid]
    if _trace:
        kernel.last_exec_time_ns = res.exec_time_ns
    return full.astype(np.float32)


# revision 27
# speedup vs baseline: 1.1772x; 1.0175x over previous
"""CloudResourceGNN (2-layer GAT + resource embedding) on 8 Trainium2 NeuronCores.

Layout: nodes sorted by in-degree (desc) and dealt into 128-node blocks;
global block g -> core g%8, slot g//8, partition = n%128. Edges grouped by
dst block; slot (tile i, partition p) holds the i-th edge (src-sorted) of
dst node p in the block, padded to the block's max degree. With partition ==
dst: a_dst is a per-partition Activation-engine bias, the softmax scatter is
an identity-lhsT PSUM accumulate, and no per-edge dst-side gathers exist.
Src rows are fetched with one SWDGE dma_gather per (layer, block) from
pair-packed tables (1024B rows L1 / 512B L2, idx = src>>1, int16-safe);
parity is resolved by dual masked matmuls. Softmax runs without
max-subtraction: w = exp(leaky_relu(asrc+adst)) with denominators from ones
columns. LayerNorm runs on the Activation engine via ln/exp (one act table).
The graph-independent resource-embedding half of the output is computed and
written during the layer-1 edge phase.
"""

import numpy as np
import ml_dtypes

import concourse.bass as bass
import concourse.bacc as bacc
import concourse.mybir as mybir
import concourse.tile as tile

BF16 = mybir.dt.bfloat16
F32 = mybir.dt.float32
I16 = mybir.dt.int16
OPc = mybir.AluOpType
AF = mybir.ActivationFunctionType
nbf = ml_dtypes.bfloat16

NEG_SLOPE = 0.2
LN_EPS = 1e-5
P = 128


class Geo:
    pass


def _wrap16(vals):
    """idx list (len % 128 == 0) -> [128, n/16] wrapped-16, replicated x8."""
    v = np.asarray(vals, np.int64)
    assert len(v) % 128 == 0
    w = v.reshape(-1, 16).T                    # [16, n/16]
    return np.tile(w, (8, 1)).astype(np.int16)  # [128, n/16]


def build_geometry(N, n_cores, src, dst, cap=20):
    g = Geo()
    g.N = N
    g.n_cores = n_cores
    per_core_nodes = -(-N // n_cores)
    g.nblk = -(-per_core_nodes // P)
    g.npc = g.nblk * P
    g.node_pad = g.npc * n_cores
    g.ntile = g.node_pad // P          # global blocks

    loop = np.arange(N, dtype=np.int64)
    s_all = np.concatenate([np.asarray(src, np.int64), loop])
    d_all = np.concatenate([np.asarray(dst, np.int64), loop])
    deg = np.bincount(d_all, minlength=N)

    # degree-sorted placement: new id n = rank in descending-degree order
    g.order = np.argsort(-deg, kind="stable")          # new -> old
    pos = np.empty(N, np.int64)
    pos[g.order] = np.arange(N)                        # old -> new
    g.pos = pos

    sn = pos[s_all]
    dn = pos[d_all]
    gblk = dn // P                                     # global dst block
    core = gblk % n_cores
    lblk = gblk // n_cores                             # local block slot
    part = dn % P

    # tiles per local-block slot j: max over cores/partitions of per-node deg
    degn = np.zeros(g.node_pad, np.int64)
    degn[:N] = deg[g.order]
    dmax = degn.reshape(g.ntile, P).max(axis=1)        # per global block
    g.Tb = np.zeros(g.nblk, np.int64)
    for j in range(g.nblk):
        g.Tb[j] = max(1, dmax[j * n_cores:(j + 1) * n_cores].max())
    g.T = int(g.Tb.sum())
    g.S = g.T * P
    g.t0 = np.zeros(g.nblk, np.int64)
    g.t0[1:] = np.cumsum(g.Tb)[:-1]
    g.gmax = min(cap, int(g.Tb.max()))
    # chunks: (block, tile_lo, tile_hi, first, last) with tile_hi-tile_lo <= cap
    g.chunks = []
    for j in range(g.nblk):
        lo = 0
        while lo < g.Tb[j]:
            hi = min(lo + cap, int(g.Tb[j]))
            g.chunks.append((j, lo, hi, lo == 0, hi == g.Tb[j]))
            lo = hi

    # layer-2 global row of node n (AllGather order: core-major)
    def t2row(n):
        gb = n // P
        return (gb % n_cores) * g.npc + (gb // n_cores) * P + n % P

    g.ix1 = np.zeros((n_cores, P, g.S // 16), np.int16)
    g.ix2 = np.zeros((n_cores, P, g.S // 16), np.int16)
    g.jm = np.zeros((n_cores, P, 2 * g.T), np.float32)
    g.ownmask = np.zeros((n_cores, P, g.ntile), np.float32)

    for k in range(n_cores):
        m = core == k
        s, j, p = sn[m], lblk[m], part[m]
        # sort edges by (block, partition, src) so slot i of node p is its
        # i-th smallest src (quantile locality across partitions)
        o = np.lexsort((s, p, j))
        s, j, p = s[o], j[o], p[o]
        v1 = np.zeros(g.S, np.int64)
        v2 = np.zeros(g.S, np.int64)
        pme = np.zeros(g.S, np.float32)
        pmo = np.zeros(g.S, np.float32)
        vld = np.zeros(g.S, bool)
        # slot index: i-th edge of (j, p) -> (g.t0[j] + i) * P + p
        # compute i via cumcount within (j, p) runs
        if len(s):
            key = j * P + p
            start = np.r_[0, np.nonzero(np.diff(key))[0] + 1]
            runlen = np.diff(np.r_[start, len(key)])
            within = np.arange(len(key)) - np.repeat(start, runlen)
            slot = (g.t0[j] + within) * P + p
            v1[slot] = s >> 1
            v2[slot] = t2row(s) >> 1
            vld[slot] = True
            even = (s % 2 == 0).astype(np.float32)
            pme[slot] = even
            pmo[slot] = 1.0 - even
        # trailing-pad slots per chunk -> idx -1 (Q7 trims them)
        for (j2, tl, th, _, _) in g.chunks:
            a, b = (g.t0[j2] + tl) * P, (g.t0[j2] + th) * P
            e = b
            while e > a and not vld[e - 1]:
                e -= 1
            g.ix1[k, :, a // 16:b // 16] = _wrap16(v1[a:b])
            g.ix2[k, :, a // 16:b // 16] = _wrap16(v2[a:b])
        jm = np.stack([pme, pmo], axis=-1).reshape(g.T, P, 2)
        g.jm[k] = jm.transpose(1, 0, 2).reshape(P, 2 * g.T)
        own = np.zeros(g.ntile, np.float32)
        own[np.arange(g.ntile) % n_cores == k] = 1.0
        g.ownmask[k] = np.tile(own, (P, 1))
    return g


def pack_weights(W1, att_src1, att_dst1, W2, att_src2, att_dst2, hid, heads):
    C1 = W1.shape[0]
    n1 = 2 * (hid + 1) + 2 * heads      # 134: [h0|1|h1|1|as0,as1|ad0,ad1]
    rhs1 = np.zeros((C1, n1), dtype=np.float32)
    rhs1[:, 0:hid] = W1[:, 0:hid]
    rhs1[:, hid + 1:2 * hid + 1] = W1[:, hid:2 * hid]
    Wh = W1.reshape(C1, heads, hid)
    rhs1[:, 2 * hid + 2:2 * hid + 2 + heads] = np.einsum("ihc,hc->ih", Wh, att_src1)
    rhs1[:, 2 * hid + 2 + heads:] = np.einsum("ihc,hc->ih", Wh, att_dst1)
    ones1 = np.zeros((1, n1), dtype=np.float32)
    ones1[0, hid] = 1.0
    ones1[0, 2 * hid + 1] = 1.0
    C2 = W2.shape[0]
    n2 = hid + 3                        # 67: [h|1|asrc2|adst2]
    rhs2 = np.zeros((C2, n2), dtype=np.float32)
    rhs2[:, 0:hid] = W2
    rhs2[:, hid + 1] = W2 @ att_src2[0]
    rhs2[:, hid + 2] = W2 @ att_dst2[0]
    ones2 = np.zeros((1, n2), dtype=np.float32)
    ones2[0, hid] = 1.0
    return rhs1, ones1, rhs2, ones2


def build_program(g, hid=64, heads=2, C1=128, R=16, res_dim=64):
    NT = g.ntile
    NB = g.nblk
    n1 = 2 * (hid + 1) + 2 * heads      # 134
    n2 = hid + 3                        # 67
    w1c = hid + 1                       # 65
    T1C = 256                           # table1 cols per node (bf16, 512B)
    T2C = 128                           # table2 cols per node (bf16, 256B)
    RROW = NB * P * R

    nc = bacc.Bacc("TRN2", target_bir_lowering=False, debug=False,
                   num_devices=g.n_cores)

    xT_bf = nc.dram_tensor("xT_bf", [C1, g.node_pad], BF16, kind="ExternalInput")
    rhs1_d = nc.dram_tensor("rhs1", [C1, n1], BF16, kind="ExternalInput")
    ones1_d = nc.dram_tensor("ones1row", [1, n1], BF16, kind="ExternalInput")
    rhs2_d = nc.dram_tensor("rhs2", [C1, n2], BF16, kind="ExternalInput")
    ones2_d = nc.dram_tensor("ones2row", [1, n2], BF16, kind="ExternalInput")
    resw_d = nc.dram_tensor("resw", [res_dim + 1, hid], BF16, kind="ExternalInput")
    consts_d = nc.dram_tensor("consts", [8, 512], F32, kind="ExternalInput")
    ident_d = nc.dram_tensor("ident", [P, P], BF16, kind="ExternalInput")
    resT_d = nc.dram_tensor("resT_bf", [res_dim, RROW], BF16, kind="ExternalInput")
    ix1_d = nc.dram_tensor("ix1", [P, g.S // 16], I16, kind="ExternalInput")
    ix2_d = nc.dram_tensor("ix2", [P, g.S // 16], I16, kind="ExternalInput")
    jm_d = nc.dram_tensor("jm", [P, 2 * g.T], F32, kind="ExternalInput")
    own_d = nc.dram_tensor("ownmask", [P, NT], F32, kind="ExternalInput")
    out_d = nc.dram_tensor("out", [g.npc, R, 2 * hid], F32, kind="ExternalOutput")

    table1 = nc.dram_tensor("table1", [g.node_pad, T1C], BF16)
    myblk2 = nc.dram_tensor("myblk2", [g.npc, T2C], BF16)
    table2 = nc.dram_tensor("table2", [g.n_cores * g.npc, T2C], BF16,
                            addr_space="Shared")
    t1pair = table1.ap().rearrange("(r x) c -> r (x c)", x=2)   # [np/2, 512]
    t2pair = table2.ap().rearrange("(r x) c -> r (x c)", x=2)   # [np*8/2? -> 512]

    with tile.TileContext(nc) as tc:
        with tc.tile_pool(name="consts", bufs=1) as cpool, \
             tc.tile_pool(name="persist", bufs=1) as pp:
            crow = []
            for r in range(8):
                t_ = cpool.tile([1, 512], F32, tag=f"crow{r}", name=f"crow{r}")
                nc.sync.dma_start(out=t_[:, :], in_=consts_d[r:r + 1, :])
                crow.append(t_)
            onesbf = cpool.tile([1, P], BF16)
            nc.vector.tensor_copy(out=onesbf[:, :], in_=crow[7][:, 0:P])
            eps_t = cpool.tile([P, 1], F32)
            nc.vector.memset(eps_t[:, :], LN_EPS)
            grd_t = cpool.tile([P, 1], F32)
            nc.vector.memset(grd_t[:, :], 1e-20)
            ident_s = cpool.tile([P, P], BF16)
            nc.sync.dma_start(out=ident_s[:, :], in_=ident_d[:, :])
            rhs1_s = cpool.tile([C1, n1], BF16)
            nc.sync.dma_start(out=rhs1_s[:, :], in_=rhs1_d[:, :])
            ones1_s = cpool.tile([1, n1], BF16)
            nc.sync.dma_start(out=ones1_s[:, :], in_=ones1_d[:, :])
            rhs2_s = cpool.tile([C1, n2], BF16)
            nc.sync.dma_start(out=rhs2_s[:, :], in_=rhs2_d[:, :])
            ones2_s = cpool.tile([1, n2], BF16)
            nc.sync.dma_start(out=ones2_s[:, :], in_=ones2_d[:, :])
            resw_s = cpool.tile([res_dim + 1, hid], BF16)
            nc.sync.dma_start(out=resw_s[:, :], in_=resw_d[:, :])

            ones_f = cpool.tile([1, P], F32)
            nc.vector.tensor_copy(out=ones_f[:, :], in_=crow[7][:, 0:P])
            b1_rep = cpool.tile([P, 2 * hid], F32)
            b2_rep = cpool.tile([P, hid], F32)
            lnw_rep = cpool.tile([P, hid], F32)
            lnb_rep = cpool.tile([P, hid], F32)
            with tc.tile_pool(name="repl_ps", bufs=2, space="PSUM") as rps:
                for dst_t, row, ncol in (
                    (b1_rep, 0, 2 * hid), (b2_rep, 1, hid),
                    (lnw_rep, 2, hid), (lnb_rep, 3, hid),
                ):
                    pst = rps.tile([P, 512], F32, tag="repl", name=f"repl{row}")
                    nc.tensor.matmul(out=pst[:, 0:ncol], lhsT=ones_f[:, :],
                                     rhs=crow[row][:, 0:ncol],
                                     start=True, stop=True)
                    nc.vector.tensor_copy(out=dst_t[:, 0:ncol],
                                          in_=pst[:, 0:ncol])

            jm_sb = pp.tile([P, g.T, 2], F32)
            nc.sync.dma_start(out=jm_sb[:, :, :], in_=jm_d[:, :])
            ix1_sb = pp.tile([P, g.S // 16], I16)
            nc.sync.dma_start(out=ix1_sb[:, :], in_=ix1_d[:, :])
            ix2_sb = pp.tile([P, g.S // 16], I16)
            nc.sync.dma_start(out=ix2_sb[:, :], in_=ix2_d[:, :])
            ownm = pp.tile([P, NT], F32)
            nc.sync.dma_start(out=ownm[:, :], in_=own_d[:, :])
            blk2_sb = pp.tile([P, NB, T2C], BF16)
            adst1_sb = pp.tile([P, NB, heads], F32)
            adst2_sb = pp.tile([P, NB, 1], F32)
            nc.vector.memset(adst1_sb[:, :, :], 0.0)
            nc.vector.memset(adst2_sb[:, :, :], 0.0)
            nc.vector.memset(blk2_sb[:, :, :], 0.0)

            # ---------------- phase 1: node phase (replicated) -------------
            XCH = 32
            with tc.tile_pool(name="n1_xt", bufs=2) as xtp, \
                 tc.tile_pool(name="n1_ps", bufs=4, space="PSUM") as n1ps, \
                 tc.tile_pool(name="n1_st", bufs=3) as n1st:
                nch = -(-NT // XCH)
                sb_iter = 0
                for c in range(nch):
                    tn0 = c * XCH
                    ntl = min(XCH, NT - tn0)
                    xt = xtp.tile([P, XCH * P], BF16, tag="xt")
                    nc.sync.dma_start(out=xt[:, 0:ntl * P],
                                      in_=xT_bf[:, tn0 * P:(tn0 + ntl) * P])
                    nst = -(-ntl // 4)
                    for sb in range(nst):
                        st = n1st.tile([P, 4, T1C], BF16, tag="n1st")
                        nn = min(4, ntl - sb * 4)
                        if sb_iter < 3:
                            nc.vector.memset(st[:, :, n1 - 2:T1C], 0.0)
                        sb_iter += 1
                        for i in range(nn):
                            t = sb * 4 + i
                            gt_ = tn0 + t
                            b = gt_ // g.n_cores
                            ps = n1ps.tile([P, n1], F32, tag="n1ps")
                            nc.tensor.matmul(out=ps[:, :],
                                             lhsT=xt[:, t * P:(t + 1) * P],
                                             rhs=rhs1_s[:, :],
                                             start=True, stop=False)
                            nc.tensor.matmul(out=ps[:, :], lhsT=onesbf[:, :],
                                             rhs=ones1_s[:, :],
                                             start=False, stop=True)
                            nc.scalar.copy(out=st[:, i:i + 1, 0:n1 - 2],
                                           in_=ps[:, 0:n1 - 2])
                            nc.vector.scalar_tensor_tensor(
                                out=adst1_sb[:, b:b + 1, 0:heads],
                                in0=ps[:, n1 - 2:n1],
                                scalar=ownm[:, gt_:gt_ + 1],
                                in1=adst1_sb[:, b:b + 1, 0:heads],
                                op0=OPc.mult, op1=OPc.add)
                        nc.sync.dma_start(
                            out=table1.ap().rearrange(
                                "(t p) c -> p t c",
                                p=P)[:, tn0 + sb * 4:tn0 + sb * 4 + nn, :],
                            in_=st[:, 0:nn, :])

            # ------- phase 2: layer-1 edge phase + res embedding -----------
            with tc.tile_pool(name="e1_g", bufs=4) as gp, \
                 tc.tile_pool(name="e1_w", bufs=2) as wp, \
                 tc.tile_pool(name="e1_ps", bufs=3, space="PSUM") as eps, \
                 tc.tile_pool(name="e1_tp", bufs=1, space="PSUM") as tps, \
                 tc.tile_pool(name="e1_h2", bufs=1, space="PSUM") as h2ps, \
                 tc.tile_pool(name="e1_x2", bufs=2) as x2p, \
                 tc.tile_pool(name="res_t", bufs=2) as resp, \
                 tc.tile_pool(name="res_ps", bufs=2, space="PSUM") as rps2:
                res_iter = 0
                g_iter = 0
                psum_cur = {}
                for ci, (b, tl, th, first, last) in enumerate(g.chunks):
                    t0 = int(g.t0[b]) + tl
                    ntl = th - tl
                    gt = gp.tile([P, g.gmax, 2 * T1C], BF16, tag="g1",
                                 name=f"g1_{ci}")
                    if g_iter < 3:
                        nc.vector.memset(gt[:, :, :], 0.0)
                    g_iter += 1
                    nc.gpsimd.dma_gather(
                        gt[:, 0:ntl, :], t1pair,
                        ix1_sb[:, t0 * 8:(t0 + ntl) * 8], ntl * P, ntl * P,
                        2 * T1C, single_packet=False)
                    # w = exp(leaky(asrc + adst)) per (parity, head)
                    wt = wp.tile([P, g.gmax, 4], F32, tag="w1", name=f"w1_{ci}")
                    for par in range(2):
                        for h in range(heads):
                            c = par * heads + h
                            ac = par * T1C + n1 - 4 + h
                            nc.scalar.activation(
                                out=wt[:, 0:ntl, c:c + 1],
                                in_=gt[:, 0:ntl, ac:ac + 1],
                                func=AF.Identity,
                                bias=adst1_sb[:, b, h:h + 1])
                    nc.vector.scalar_tensor_tensor(
                        out=wt[:, 0:ntl, :], in0=wt[:, 0:ntl, :],
                        scalar=NEG_SLOPE, in1=wt[:, 0:ntl, :],
                        op0=OPc.mult, op1=OPc.max)
                    nc.scalar.activation(out=wt[:, 0:ntl, :],
                                         in_=wt[:, 0:ntl, :], func=AF.Exp)
                    nc.vector.tensor_tensor(
                        out=wt[:, 0:ntl, 0:2], in0=wt[:, 0:ntl, 0:2],
                        in1=jm_sb[:, t0:t0 + ntl, 0:1].to_broadcast(
                            [P, ntl, 2]), op=OPc.mult)
                    nc.vector.tensor_tensor(
                        out=wt[:, 0:ntl, 2:4], in0=wt[:, 0:ntl, 2:4],
                        in1=jm_sb[:, t0:t0 + ntl, 1:2].to_broadcast(
                            [P, ntl, 2]), op=OPc.mult)
                    gs = gp.tile([P, g.gmax, 4 * w1c], BF16, tag="gs",
                                 name=f"gs_{ci}")
                    for par in range(2):
                        for h in range(heads):
                            nc.vector.tensor_tensor(
                                out=gs[:, 0:ntl,
                                       (par * heads + h) * w1c:
                                       (par * heads + h + 1) * w1c],
                                in0=gt[:, 0:ntl,
                                       par * T1C + h * w1c:
                                       par * T1C + (h + 1) * w1c],
                                in1=wt[:, 0:ntl,
                                       par * heads + h:par * heads + h + 1
                                       ].to_broadcast([P, ntl, w1c]),
                                op=OPc.mult)
                    if first:
                        psum_cur[b] = eps.tile([P, heads * w1c], F32,
                                               tag="e1ps", name=f"e1ps_{b}")
                    pc = psum_cur[b]
                    for i in range(ntl):
                        for par in range(2):
                            nc.tensor.matmul(
                                out=pc[:, :], lhsT=ident_s[:, :],
                                rhs=gs[:, i:i + 1,
                                       par * heads * w1c:
                                       (par + 1) * heads * w1c],
                                start=(first and i == 0 and par == 0),
                                stop=(last and i == ntl - 1 and par == 1))
                    if not last:
                        continue
                    pc = psum_cur.pop(b)
                    # epilogue: softmax div + bias + ELU -> x2t
                    x2pre = x2p.tile([P, 2 * hid], F32, tag="x2pre",
                                     name=f"x2pre_{b}")
                    esc = x2p.tile([P, 2 * hid], F32, tag="esc",
                                   name=f"esc_{b}")
                    x2t = x2p.tile([P, 2 * hid], BF16, tag="x2",
                                   name=f"x2_{b}")
                    for h in range(heads):
                        rec = x2p.tile([P, 1], F32, tag=f"rec{h}",
                                       name=f"rec{h}_{b}")
                        dn = x2p.tile([P, 1], F32, tag=f"dn{h}",
                                      name=f"dn{h}_{b}")
                        nc.vector.tensor_scalar(
                            out=dn[:, :], in0=pc[:, (h + 1) * w1c - 1:
                                                 (h + 1) * w1c],
                            scalar1=1e-30, scalar2=None, op0=OPc.add)
                        nc.vector.reciprocal(out=rec[:, :], in_=dn[:, :])
                        nc.vector.scalar_tensor_tensor(
                            out=x2pre[:, h * hid:(h + 1) * hid],
                            in0=pc[:, h * w1c:h * w1c + hid],
                            scalar=rec[:, 0:1],
                            in1=b1_rep[:, h * hid:(h + 1) * hid],
                            op0=OPc.mult, op1=OPc.add)
                    # ELU: x2t = max(exp(min(x,0)) - 1, x)
                    nc.scalar.activation(out=esc[:, :], in_=x2pre[:, :],
                                         func=AF.Relu, scale=-1.0)
                    nc.scalar.activation(out=esc[:, :], in_=esc[:, :],
                                         func=AF.Exp, scale=-1.0)
                    nc.vector.scalar_tensor_tensor(
                        out=x2t[:, :], in0=esc[:, :], scalar=-1.0,
                        in1=x2pre[:, :], op0=OPc.add, op1=OPc.max)
                    tp = tps.tile([P, P], BF16, tag="x2tp", name=f"tp_{b}")
                    nc.tensor.transpose(out=tp[:, :], in_=x2t[:, :],
                                        identity=ident_s[:, :])
                    x2tt = x2p.tile([P, P], BF16, tag="x2tt", name=f"x2tt_{b}")
                    nc.scalar.copy(out=x2tt[:, :], in_=tp[:, :])
                    h2 = h2ps.tile([P, n2], F32, tag="h2ps", name=f"h2_{b}")
                    nc.tensor.matmul(out=h2[:, :], lhsT=x2tt[:, :],
                                     rhs=rhs2_s[:, :], start=True, stop=False)
                    nc.tensor.matmul(out=h2[:, :], lhsT=onesbf[:, :],
                                     rhs=ones2_s[:, :], start=False, stop=True)
                    nc.scalar.copy(out=blk2_sb[:, b:b + 1, 0:n2 - 1],
                                   in_=h2[:, 0:n2 - 1])
                    nc.vector.tensor_copy(out=adst2_sb[:, b:b + 1, 0:1],
                                          in_=h2[:, n2 - 1:n2])

                    # res embedding for this block (graph-independent)
                    rt = resp.tile([res_dim + 1, P, R], BF16, tag="rest",
                                   name=f"rt_{b}")
                    if res_iter < 2:
                        nc.vector.memset(rt[res_dim:res_dim + 1, :, :], 1.0)
                    res_iter += 1
                    nc.sync.dma_start(
                        out=rt[0:res_dim, :, :],
                        in_=resT_d[:, b * P * R:(b + 1) * P * R])
                    for half in range(2):
                        rp = rps2.tile([P, 8 * hid], F32, tag="resps",
                                       name=f"rp_{b}_{half}")
                        for r8 in range(8):
                            r = half * 8 + r8
                            nc.tensor.matmul(
                                out=rp[:, r8 * hid:(r8 + 1) * hid],
                                lhsT=rt[:, :, r:r + 1],
                                rhs=resw_s[:, :], start=True, stop=True)
                        em = resp.tile([P, 8, hid], F32, tag="em",
                                       name=f"em_{b}_{half}")
                        ro = resp.tile([P, 8, hid], F32, tag="ro",
                                       name=f"ro_{b}_{half}")
                        nc.scalar.activation(out=em[:, :, :], in_=rp[:, :],
                                             func=AF.Relu, scale=-1.0)
                        nc.scalar.activation(out=em[:, :, :], in_=em[:, :, :],
                                             func=AF.Exp, scale=-1.0)
                        nc.vector.scalar_tensor_tensor(
                            out=ro[:, :, :], in0=em[:, :, :], scalar=-1.0,
                            in1=rp[:, :], op0=OPc.add, op1=OPc.max)
                        nc.sync.dma_start(
                            out=out_d[b * P:(b + 1) * P,
                                      half * 8:(half + 1) * 8, hid:2 * hid],
                            in_=ro[:, :, :])

            nc.sync.dma_start(
                out=myblk2.ap().rearrange("(j p) c -> p j c", p=P)[:, :, :],
                in_=blk2_sb[:, :, :])
            nc.gpsimd.collective_compute(
                "AllGather", OPc.bypass,
                replica_groups=[list(range(g.n_cores))],
                ins=[myblk2.ap().opt()],
                outs=[table2.ap().opt()],
            )

            # -------- phase 3: layer-2 edge phase + LN + output ------------
            GRP = 8
            with tc.tile_pool(name="e2_g", bufs=4) as gp2, \
                 tc.tile_pool(name="e2_w", bufs=2) as wp2, \
                 tc.tile_pool(name="e2_ps", bufs=3, space="PSUM") as eps2, \
                 tc.tile_pool(name="ln", bufs=2) as lnp, \
                 tc.tile_pool(name="lng", bufs=2) as lgp:
                g2_iter = 0
                psum2 = {}
                xcg = None
                for ci, (b, tl, th, first, last) in enumerate(g.chunks):
                    t0 = int(g.t0[b]) + tl
                    ntl = th - tl
                    gt2 = gp2.tile([P, g.gmax, 2 * T2C], BF16, tag="g2",
                                   name=f"g2_{ci}")
                    if g2_iter < 3:
                        nc.vector.memset(gt2[:, :, :], 0.0)
                    g2_iter += 1
                    nc.gpsimd.dma_gather(
                        gt2[:, 0:ntl, :], t2pair,
                        ix2_sb[:, t0 * 8:(t0 + ntl) * 8], ntl * P, ntl * P,
                        2 * T2C, single_packet=False)
                    wt2 = wp2.tile([P, g.gmax, 2], F32, tag="w2",
                                   name=f"w2_{ci}")
                    for par in range(2):
                        ac = par * T2C + hid + 1
                        nc.scalar.activation(
                            out=wt2[:, 0:ntl, par:par + 1],
                            in_=gt2[:, 0:ntl, ac:ac + 1],
                            func=AF.Identity,
                            bias=adst2_sb[:, b, 0:1])
                    nc.vector.scalar_tensor_tensor(
                        out=wt2[:, 0:ntl, :], in0=wt2[:, 0:ntl, :],
                        scalar=NEG_SLOPE, in1=wt2[:, 0:ntl, :],
                        op0=OPc.mult, op1=OPc.max)
                    nc.scalar.activation(out=wt2[:, 0:ntl, :],
                                         in_=wt2[:, 0:ntl, :], func=AF.Exp)
                    nc.vector.tensor_tensor(
                        out=wt2[:, 0:ntl, :], in0=wt2[:, 0:ntl, :],
                        in1=jm_sb[:, t0:t0 + ntl, :], op=OPc.mult)
                    gs2 = gp2.tile([P, g.gmax, 2 * w1c], BF16, tag="gs2",
                                   name=f"gs2_{ci}")
                    for par in range(2):
                        nc.vector.tensor_tensor(
                            out=gs2[:, 0:ntl, par * w1c:(par + 1) * w1c],
                            in0=gt2[:, 0:ntl, par * T2C:par * T2C + w1c],
                            in1=wt2[:, 0:ntl, par:par + 1].to_broadcast(
                                [P, ntl, w1c]),
                            op=OPc.mult)
                    if first:
                        psum2[b] = eps2.tile([P, w1c], F32, tag="e2ps",
                                             name=f"e2ps_{b}")
                    ps2 = psum2[b]
                    for i in range(ntl):
                        for par in range(2):
                            nc.tensor.matmul(
                                out=ps2[:, :], lhsT=ident_s[:, :],
                                rhs=gs2[:, i:i + 1,
                                        par * w1c:(par + 1) * w1c],
                                start=(first and i == 0 and par == 0),
                                stop=(last and i == ntl - 1 and par == 1))
                    if not last:
                        continue
                    ps2 = psum2.pop(b)
                    jg = b % GRP
                    if jg == 0:
                        xcg = lgp.tile([P, GRP, hid], F32, tag="xcg",
                                       name=f"xcg_{b}")
                        mvg = lgp.tile([P, GRP, 2], F32, tag="mvg",
                                       name=f"mvg_{b}")
                        sdg = lgp.tile([P, GRP], F32, tag="sdg",
                                       name=f"sdg_{b}")
                        rsg = lgp.tile([P, GRP], F32, tag="rsg",
                                       name=f"rsg_{b}")
                    # softmax div + bias -> y; mean/var on DVE (bn_stats)
                    y = lnp.tile([P, hid], F32, tag="y", name=f"y_{b}")
                    rec = lnp.tile([P, 1], F32, tag="rec2", name=f"r2_{b}")
                    dn2 = lnp.tile([P, 1], F32, tag="dn2", name=f"d2_{b}")
                    st6 = lnp.tile([P, 6], F32, tag="st6", name=f"s6_{b}")
                    nc.vector.tensor_scalar(
                        out=dn2[:, :], in0=ps2[:, hid:hid + 1],
                        scalar1=1e-30, scalar2=None, op0=OPc.add)
                    nc.vector.reciprocal(out=rec[:, :], in_=dn2[:, :])
                    nc.vector.scalar_tensor_tensor(
                        out=y[:, :], in0=ps2[:, 0:hid], scalar=rec[:, 0:1],
                        in1=b2_rep[:, :], op0=OPc.mult, op1=OPc.add)
                    nc.vector.bn_stats(out=st6[:, :], in_=y[:, :])
                    nc.vector.bn_aggr(out=mvg[:, jg, :], in_=st6[:, :])
                    nc.vector.tensor_scalar(
                        out=xcg[:, jg, :], in0=y[:, :],
                        scalar1=mvg[:, jg, 0:1], scalar2=None,
                        op0=OPc.subtract)
                    if jg == GRP - 1 or b == NB - 1:
                        gn = jg + 1
                        b0 = b - jg
                        nc.scalar.activation(out=sdg[:, 0:gn],
                                             in_=mvg[:, 0:gn, 1],
                                             func=AF.Sqrt,
                                             bias=eps_t[:, 0:1])
                        nc.vector.reciprocal(out=rsg[:, 0:gn],
                                             in_=sdg[:, 0:gn])
                        for j2 in range(gn):
                            bb = b0 + j2
                            lnh = lnp.tile([P, 1, hid], F32, tag="lnh",
                                           name=f"lnh_{bb}")
                            nc.vector.scalar_tensor_tensor(
                                out=lnh[:, 0, :], in0=xcg[:, j2, :],
                                scalar=rsg[:, j2:j2 + 1],
                                in1=lnw_rep[:, :],
                                op0=OPc.mult, op1=OPc.mult)
                            nc.vector.tensor_tensor(out=lnh[:, 0, :],
                                                    in0=lnh[:, 0, :],
                                                    in1=lnb_rep[:, :],
                                                    op=OPc.add)
                            nc.sync.dma_start(
                                out=out_d[bb * P:(bb + 1) * P, :, 0:hid],
                                in_=lnh[:, 0:1, :].to_broadcast(
                                    [P, R, hid]))
    nc.compile()
    return nc


# ----------------------------------------------------------------------------
# host wrapper
# ----------------------------------------------------------------------------

def make_inputs(g, x, resource_features, W1, att_src1, att_dst1, b1,
                W2, att_src2, att_dst2, b2, ln_w, ln_b, res_W, res_b):
    N, C1 = x.shape
    R = resource_features.shape[1]
    res_dim = resource_features.shape[2]
    heads = att_src1.shape[0]
    hid = W2.shape[1]
    rhs1, ones1, rhs2, ones2 = pack_weights(
        W1, att_src1, att_dst1, W2, att_src2, att_dst2, hid, heads)

    x_pad = np.zeros((g.node_pad, C1), dtype=np.float32)
    x_pad[:N] = x[g.order]
    xT_pad = np.ascontiguousarray(x_pad.T).astype(nbf)
    consts = np.zeros((8, 512), dtype=np.float32)
    consts[0, 0:2 * hid] = b1
    consts[1, 0:hid] = b2
    consts[2, 0:hid] = ln_w
    consts[3, 0:hid] = ln_b
    consts[7, 0:P] = 1.0
    ident = np.eye(P, dtype=np.float32).astype(nbf)
    resw65 = np.zeros((res_dim + 1, hid), dtype=np.float32)
    resw65[0:res_dim] = res_W
    resw65[res_dim] = res_b

    res_perm = resource_features[g.order].reshape(N * R, res_dim)
    RROW = g.npc * R

    common = {
        "xT_bf": xT_pad,
        "rhs1": rhs1.astype(nbf), "ones1row": ones1.astype(nbf),
        "rhs2": rhs2.astype(nbf), "ones2row": ones2.astype(nbf),
        "resw": resw65.astype(nbf),
        "consts": consts, "ident": ident,
    }
    in_maps = []
    for k in range(g.n_cores):
        # core k owns new-ids n with (n//P) % n_cores == k, in (j, p) order
        jj = np.arange(g.npc)
        n_ids = ((jj // P) * g.n_cores + k) * P + jj % P
        valid = n_ids < N
        rc = np.zeros((RROW, res_dim), dtype=np.float32)
        rows = np.repeat(jj[valid], R) * R + np.tile(np.arange(R),
                                                     valid.sum())
        src_rows = np.repeat(n_ids[valid], R) * R + np.tile(
            np.arange(R), valid.sum())
        rc[rows] = res_perm[src_rows]
        in_maps.append(dict(
            common,
            resT_bf=np.ascontiguousarray(rc.T).astype(nbf),
            ix1=g.ix1[k], ix2=g.ix2[k], jm=g.jm[k], ownmask=g.ownmask[k],
        ))
    return in_maps


def _install_ntff_hook():
    import sys, types, contextlib, ctypes
    if "antenv.axon_hooks" in sys.modules:
        return
    so_path = "/opt/axon/libaxon_pjrt.so"
    mod = types.ModuleType("antenv.axon_hooks")
    _h = [None]
    mod.set_axon_ntff_profile_hook = lambda h: _h.__setitem__(0, h)
    mod.get_axon_ntff_profile_hook = lambda: _h[0]
    sys.modules["antenv.axon_hooks"] = mod
    try:
        lib = ctypes.CDLL(so_path)
        if not hasattr(lib, "axon_start_nrt_profile"):
            return
        lib.axon_start_nrt_profile.argtypes = [
            ctypes.POINTER(ctypes.c_int64), ctypes.c_size_t]
        lib.axon_start_nrt_profile.restype = ctypes.c_int64
        lib.axon_stop_nrt_profile.argtypes = [ctypes.c_char_p]
        lib.axon_stop_nrt_profile.restype = ctypes.c_int64

        @contextlib.contextmanager
        def _hook(output_dir, device_ids):
            import jax
            jax.devices()
            if device_ids:
                ids = (ctypes.c_int64 * len(device_ids))(*device_ids)
                rc = lib.axon_start_nrt_profile(ids, len(device_ids))
            else:
                rc = lib.axon_start_nrt_profile(None, 0)
            if rc != 0:
                raise RuntimeError(f"axon_start_nrt_profile rc={rc}")
            try:
                yield
            finally:
                n = lib.axon_stop_nrt_profile(str(output_dir).encode())
                print(f"ntff profile: {n} file(s) -> {output_dir}")

        mod.set_axon_ntff_profile_hook(_hook)
    except Exception as e:
        print("ntff hook install failed:", e)


_CACHE = {}


def kernel(x, edge_index, resource_features, W1, att_src1, att_dst1, b1,
           W2, att_src2, att_dst2, b2, ln_w, ln_b, res_W, res_b, *,
           n_cores=8, _trace=False):
    from concourse.bass_utils import run_bass_kernel_spmd
    if _trace:
        _install_ntff_hook()

    x = np.asarray(x, np.float32)
    edge_index = np.asarray(edge_index)
    resource_features = np.asarray(resource_features, np.float32)
    N, C1 = x.shape
    R = resource_features.shape[1]
    res_dim = resource_features.shape[2]
    att_src1 = np.asarray(att_src1, np.float32)
    heads = att_src1.shape[0]
    W2 = np.asarray(W2, np.float32)
    hid = W2.shape[1]

    key = ("prog", N, edge_index.shape[1])
    if key in _CACHE:
        g, nc = _CACHE[key]
    else:
        g = build_geometry(N, n_cores, edge_index[0], edge_index[1])
        nc = build_program(g, hid=hid, heads=heads, C1=C1, R=R,
                           res_dim=res_dim)
        _CACHE[key] = (g, nc)

    in_maps = make_inputs(
        g, x, resource_features, np.asarray(W1, np.float32), att_src1,
        np.asarray(att_dst1, np.float32), np.asarray(b1, np.float32),
        W2, np.asarray(att_src2, np.float32), np.asarray(att_dst2, np.float32),
        np.asarray(b2, np.float32), np.asarray(ln_w, np.float32),
        np.asarray(ln_b, np.float32), np.asarray(res_W, np.float32),
        np.asarray(res_b, np.float32))

    res = run_bass_kernel_spmd(nc, in_maps, list(range(n_cores)),
                               trace=_trace)
    outs = [np.asarray(res.results[k]["out"]) for k in range(n_cores)]
    full = np.zeros((N, R, 2 * hid), dtype=np.float32)
    for k in range(n_cores):
        jj = np.arange(g.npc)
        n_ids = ((jj // P) * n_cores + k) * P + jj % P
        valid = n_ids < N
        full[g.order[n_ids[valid]]] = outs[k][valid]
    if _trace:
        kernel.last_exec_time_ns = res.exec_time_ns
    return full.astype(np.float32)
